# revision 1
# baseline (speedup 1.0000x reference)
"""nn_AttentionAverageStdScalingModule kernel.

Self-contained: takes FULL unsharded inputs, returns FULL output.
Shapes are hardcoded per the problem spec:
  test_scores  (1, 16, 88, 88)   f32
  train_labels (30, 16, 88, 88)  f32
  test_feat    (1, 16, 256, 22, 22)  f32
  train_feats  (30, 16, 256, 22, 22) f32
  softmax_temp (1,) f32

The nseq axis (16) is the independent axis; work is processed per-sequence
(the 8-way nseq sharding used for the device path maps 2 sequences per core).
This implementation computes the full pipeline in fp32:
  cosine similarity -> temperature softmax over each memory's pixels ->
  label aggregation -> bilinear 22->88 upsample -> cross-memory mean/std ->
  exp certainty rescale + residual.
"""

import numpy as np

NMEM, NSEQ, C, WF, HF = 30, 16, 256, 22, 22
WL, HL = 88, 88
P2 = WF * HF
ALPHA = 20.0


def _resize_matrix(n_in: int, n_out: int) -> np.ndarray:
    """Row-stochastic 1-D bilinear resize matrix matching
    jax.image.resize(method='bilinear', antialias=False) semantics
    (half-pixel centers, out-of-range taps dropped and renormalized)."""
    M = np.zeros((n_out, n_in), np.float64)
    scale = n_in / n_out
    for i in range(n_out):
        x = (i + 0.5) * scale - 0.5
        x0 = int(np.floor(x))
        for tap, w in ((x0, 1.0 - (x - x0)), (x0 + 1, x - x0)):
            if 0 <= tap < n_in and w > 0.0:
                M[i, tap] += w
        s = M[i].sum()
        if s > 0:
            M[i] /= s
    return M.astype(np.float32)


_DN = _resize_matrix(WL, WF)   # (22, 88) downsample
_UP = _resize_matrix(WF, WL)   # (88, 22) upsample


def kernel(test_scores, train_labels, test_feat, train_feats, softmax_temp):
    test_scores = np.asarray(test_scores, np.float32)
    train_labels = np.asarray(train_labels, np.float32)
    test_feat = np.asarray(test_feat, np.float32)
    train_feats = np.asarray(train_feats, np.float32)
    temp = np.float32(np.asarray(softmax_temp).reshape(-1)[0])

    # Downsample labels to feature resolution: (30, 16, 22, 22) -> flat (30,16,484)
    labels_down = np.einsum(
        'ij,mnjk,lk->mnil', _DN, train_labels, _DN, optimize=True
    ).reshape(NMEM, NSEQ, P2)

    out = np.empty((1, NSEQ, WL, HL), np.float32)

    for s in range(NSEQ):
        te = test_feat[0, s].reshape(C, P2)                    # (256, 484)
        tr = train_feats[:, s].transpose(1, 0, 2, 3).reshape(C, NMEM * P2)

        te_n = te / np.sqrt((te * te).sum(axis=0, keepdims=True))
        tr_n = tr / np.sqrt((tr * tr).sum(axis=0, keepdims=True))

        # cosine similarity (484 test pixels x 30*484 memory pixels)
        sim = te_n.T @ tr_n                                     # (484, 30*484)
        sim = sim.reshape(P2, NMEM, P2).transpose(1, 0, 2)      # (30, 484, 484)

        z = temp * sim
        z -= z.max(axis=2, keepdims=True)
        ez = np.exp(z, dtype=np.float32)
        p = ez / ez.sum(axis=2, keepdims=True)                  # (30, 484, 484)

        pmt_down = np.einsum('mjk,mk->mj', p, labels_down[:, s], optimize=True)
        pmt_down = pmt_down.reshape(NMEM, WF, HF)

        # bilinear upsample 22x22 -> 88x88 for each memory
        pmt = np.einsum('ij,mjk,lk->mil', _UP, pmt_down, _UP, optimize=True)

        mean = pmt.mean(axis=0)
        std = pmt.std(axis=0, ddof=1)
        certainty = np.exp(ALPHA / (1.0 + std * std) - ALPHA)
        out[0, s] = certainty * mean + test_scores[0, s]

    return out



# revision 10
# speedup vs baseline: 2578.5683x; 2578.5683x over previous
"""nn_AttentionAverageStdScalingModule — Trainium2 Bass/Tile kernel.

Contract: kernel(**inputs) takes FULL unsharded inputs and returns the FULL
output (1, 16, 88, 88) f32.  The nseq axis (16) is sharded 2-per-core across
8 NeuronCores; each core runs an identical program on its 2 sequences.

Math per (seq s):
  te_n   = softmax_temp * test_feat[:,s]/||cols||          (256, 484)
  for each memory m:   simT[j,i] = sum_c tr[c,j]*te_n[c,i]  (484j, 484i)
      ez = exp(simT * inv_norm_tr[j])     <- per-partition scale on ScalarE
      [num;den][i] = [labels_down_m; ones]^T @ ez           (2, 484)
      pmt_down_m = num/den
  pmt_m = UP @ pmt_down_m @ UP^T  (88, 88) bilinear upsample (as matmuls)
  mean/std over m, certainty = exp(A/(1+std^2)-A)
  out = certainty*mean + test_scores

Everything runs on-device; the host only slices inputs per core and
concatenates outputs.
"""

import numpy as np

NMEM, NSEQ, C, WF, HF = 30, 16, 256, 22, 22
WL, HL = 88, 88
P2 = WF * HF            # 484
NCORES = 8
SEQ_LOC = NSEQ // NCORES  # 2
ALPHA = 20.0
JC = [128, 128, 128, 100]  # j-chunk sizes covering 484


def _resize_matrix(n_in: int, n_out: int) -> np.ndarray:
    """Row-stochastic 1-D bilinear resize matrix (half-pixel centers,
    out-of-range taps dropped + renormalized) matching
    jax.image.resize(method='bilinear', antialias=False)."""
    M = np.zeros((n_out, n_in), np.float64)
    scale = n_in / n_out
    for i in range(n_out):
        x = (i + 0.5) * scale - 0.5
        x0 = int(np.floor(x))
        for tap, w in ((x0, 1.0 - (x - x0)), (x0 + 1, x - x0)):
            if 0 <= tap < n_in and w > 0.0:
                M[i, tap] += w
        s = M[i].sum()
        if s > 0:
            M[i] /= s
    return M.astype(np.float32)


_DN = _resize_matrix(WL, WF)   # (22, 88)  downsample
_UP = _resize_matrix(WF, WL)   # (88, 22)  upsample

_CACHE = {}


def _build():
    import concourse.bass as bass
    import concourse.bacc as bacc
    import concourse.mybir as mybir
    from concourse import tile

    f32 = mybir.dt.float32
    bf16 = mybir.dt.bfloat16
    AF = mybir.ActivationFunctionType
    ALU = mybir.AluOpType
    AX = mybir.AxisListType

    nc = bacc.Bacc("TRN2", target_bir_lowering=False, debug=False,
                   num_devices=NCORES)

    # ---- I/O ----
    t_scores = nc.dram_tensor("t_scores", [SEQ_LOC, WL, HL], f32,
                              kind="ExternalInput")
    t_labels = nc.dram_tensor("t_labels", [NMEM, SEQ_LOC, WL, HL], f32,
                              kind="ExternalInput")
    t_tefeat = nc.dram_tensor("t_tefeat", [SEQ_LOC, C, WF, HF], f32,
                              kind="ExternalInput")
    t_trfeat = nc.dram_tensor("t_trfeat", [NMEM, SEQ_LOC, C, WF, HF], f32,
                              kind="ExternalInput")
    t_temp = nc.dram_tensor("t_temp", [1], f32, kind="ExternalInput")
    t_dnrt = nc.dram_tensor("t_dnrt", [WL, WF], f32, kind="ExternalInput")
    t_upt32 = nc.dram_tensor("t_upt32", [WF, WL], f32, kind="ExternalInput")
    t_upt16 = nc.dram_tensor("t_upt16", [WF, WL], bf16, kind="ExternalInput")
    t_out = nc.dram_tensor("t_out", [SEQ_LOC, WL, HL], f32,
                           kind="ExternalOutput")

    with tile.TileContext(nc) as tc:
        with (
            tc.tile_pool(name="const", bufs=1) as cpool,
            tc.tile_pool(name="seq", bufs=1) as spool,
            tc.tile_pool(name="big", bufs=2) as bpool,
            tc.tile_pool(name="ez", bufs=3) as ezpool,
            tc.tile_pool(name="sm", bufs=2) as smpool,
            tc.tile_pool(name="dram", bufs=1, space="DRAM") as dpool,
            tc.tile_pool(name="psA", bufs=2, space="PSUM") as psA,
            tc.tile_pool(name="psB", bufs=2, space="PSUM") as psB,
            tc.tile_pool(name="psC", bufs=2, space="PSUM") as psC,
            tc.tile_pool(name="psD", bufs=1, space="PSUM") as psD,
        ):
            # ---- constants ----
            ones_col32 = cpool.tile([128, 1], f32)
            nc.vector.memset(ones_col32[:], 1.0)
            ones_col16 = cpool.tile([128, 1], bf16)
            nc.vector.memset(ones_col16[:], 1.0)
            ones_row32 = cpool.tile([1, 128], f32)
            nc.vector.memset(ones_row32[:], 1.0)
            dnrt = cpool.tile([WL, WF], f32)       # DN^T (88, 22)
            nc.sync.dma_start(dnrt[:], t_dnrt[:])
            upt32 = cpool.tile([WF, WL], f32)      # UP^T (22, 88)
            nc.sync.dma_start(upt32[:], t_upt32[:])
            upt16 = cpool.tile([WF, WL], bf16)
            nc.sync.dma_start(upt16[:], t_upt16[:])
            temp_t = cpool.tile([1, 1], f32)
            nc.sync.dma_start(temp_t[:], t_temp[:])

            for s in range(SEQ_LOC):
                # =========== test-feature prep ===========
                te32 = spool.tile([128, 2, P2], f32, tag="te32")
                for h in range(2):
                    nc.sync.dma_start(
                        te32[:, h, :],
                        t_tefeat[s, 128 * h:128 * (h + 1)]
                        .rearrange("c w hh -> c (w hh)"))
                sq_te = spool.tile([128, 2, P2], f32, tag="sq_te")
                nc.vector.tensor_tensor(
                    out=sq_te[:], in0=te32[:], in1=te32[:], op=ALU.mult)
                nsq_te = psB.tile([1, P2], f32, tag="psb")
                for h in range(2):
                    nc.tensor.matmul(nsq_te[:], ones_col32[:], sq_te[:, h, :],
                                     start=(h == 0), stop=(h == 1))
                nrm_te = spool.tile([1, P2], f32, tag="nrm_te")
                nc.scalar.sqrt(nrm_te[:], nsq_te[:])
                inv_te = spool.tile([1, P2], f32, tag="inv_te")
                nc.vector.reciprocal(inv_te[:], nrm_te[:])
                tsc = spool.tile([1, P2], f32, tag="tsc")
                nc.vector.tensor_scalar_mul(tsc[:], inv_te[:],
                                            temp_t[0:1, 0:1])
                tsc_b = psA.tile([128, P2], f32, tag="psa")
                nc.tensor.matmul(tsc_b[:], ones_row32[:], tsc[:])
                te_n = spool.tile([128, 2, P2], bf16, tag="te_n")
                for h in range(2):
                    nc.vector.tensor_tensor(
                        out=te_n[:, h, :], in0=te32[:, h, :], in1=tsc_b[:],
                        op=ALU.mult)

                # =========== labels: bilinear 88->22, flatten j-major ===========
                ld_stage = dpool.tile([NMEM, P2], bf16, tag="ld_stage")
                ldsb = spool.tile([WF, NMEM, WF], bf16, tag="ldsb")
                for m in range(NMEM):
                    lab = smpool.tile([WL, HL], f32, tag="lab")
                    nc.sync.dma_start(lab[:], t_labels[m, s])
                    b1 = psB.tile([WL, WF], f32, tag="psb")
                    nc.tensor.matmul(b1[:], lab[:], dnrt[:])
                    b1s = smpool.tile([WL, WF], f32, tag="b1s")
                    nc.scalar.copy(b1s[:], b1[:])
                    ldp = psB.tile([WF, WF], f32, tag="psb")
                    nc.tensor.matmul(ldp[:], b1s[:], dnrt[:])
                    nc.vector.tensor_copy(ldsb[:, m, :], ldp[:])
                nc.sync.dma_start(
                    ld_stage[:].rearrange("m (i k) -> i m k", k=WF), ldsb[:])
                # weights for aggregation: [:, m, q, 0]=labels, [:, m, q, 1]=1
                ldj = spool.tile([128, NMEM, 4, 2], bf16, tag="ldj")
                nc.vector.memset(ldj[:], 0.0)
                nc.vector.memset(ldj[:, :, 0:3, 1], 1.0)
                nc.vector.memset(ldj[0:100, :, 3, 1], 1.0)
                for q in range(4):
                    pq = JC[q]
                    nc.sync.dma_start(
                        ldj[0:pq, :, q, 0],
                        ld_stage[:, 128 * q:128 * q + pq]
                        .rearrange("m p -> p m"))

                # staging for per-memory inverse-norm relayout
                nrm_stage = dpool.tile([NMEM, 512], f32, tag="nrm_stage")
                invj = spool.tile([128, NMEM, 4], f32, tag="invj")
                ndall = spool.tile([2, NMEM, P2], f32, tag="ndall")

                # =========== per-memory pipeline ===========
                for m in range(NMEM):
                    tr32 = bpool.tile([128, 2, P2], f32, tag="tr32")
                    for h in range(2):
                        nc.sync.dma_start(
                            tr32[:, h, :],
                            t_trfeat[m, s, 128 * h:128 * (h + 1)]
                            .rearrange("c w hh -> c (w hh)"))
                    trbf = bpool.tile([128, 2, P2], bf16, tag="trbf")
                    nc.vector.tensor_scalar_mul(trbf[:], tr32[:], 1.0)
                    sqbf = bpool.tile([128, 2, P2], bf16, tag="sqbf")
                    nc.vector.tensor_tensor(
                        out=sqbf[:], in0=trbf[:], in1=trbf[:], op=ALU.mult)
                    nsq = psC.tile([1, P2], f32, tag="psc")
                    for h in range(2):
                        nc.tensor.matmul(nsq[:], ones_col16[:], sqbf[:, h, :],
                                         start=(h == 0), stop=(h == 1))
                    nrm = smpool.tile([1, P2], f32, tag="nrm")
                    nc.scalar.sqrt(nrm[:], nsq[:])
                    invr = smpool.tile([1, P2], f32, tag="invr")
                    nc.vector.reciprocal(invr[:], nrm[:])
                    nc.sync.dma_start(nrm_stage[m, 0:P2], invr[:])
                    nc.sync.dma_start(
                        invj[:, m, :],
                        nrm_stage[m].rearrange("(q p) -> p q", p=128))

                    ag = psB.tile([2, P2], f32, tag="psb")
                    for q in range(4):
                        pq = JC[q]
                        j0 = 128 * q
                        st = psA.tile([128, P2], f32, tag="psa")
                        for h in range(2):
                            nc.tensor.matmul(
                                st[0:pq, :], trbf[:, h, j0:j0 + pq],
                                te_n[:, h, :],
                                start=(h == 0), stop=(h == 1))
                        ez = ezpool.tile([128, P2], bf16, tag="ez")
                        nc.scalar.activation(
                            ez[0:pq, :], st[0:pq, :], AF.Exp,
                            scale=invj[0:pq, m, q:q + 1])
                        nc.tensor.matmul(
                            ag[:], ldj[0:pq, m, q, :], ez[0:pq, :],
                            start=(q == 0), stop=(q == 3))
                    nc.scalar.copy(ndall[:, m, :], ag[:])

                # =========== batched num/den division ===========
                nd_stage = dpool.tile([2, NMEM, P2], f32, tag="nd_stage")
                nc.sync.dma_start(nd_stage[:], ndall[:])
                numt = spool.tile([121, 120], f32, tag="numt")
                nc.sync.dma_start(
                    numt[:], nd_stage[0].rearrange("m j -> (m j)")
                    .rearrange("(p x) -> p x", p=121))
                dent = spool.tile([121, 120], f32, tag="dent")
                nc.sync.dma_start(
                    dent[:], nd_stage[1].rearrange("m j -> (m j)")
                    .rearrange("(p x) -> p x", p=121))
                rden = spool.tile([121, 120], f32, tag="rden")
                nc.vector.reciprocal(rden[:], dent[:])
                pdq = spool.tile([121, 120], f32, tag="pdq")
                nc.vector.tensor_tensor(out=pdq[:], in0=numt[:], in1=rden[:],
                                        op=ALU.mult)
                # =========== upsample + stats + output ===========
                pd_stage = dpool.tile([NMEM, P2], f32, tag="pd_stage")
                nc.sync.dma_start(
                    pd_stage[:].rearrange("m j -> (m j)")
                    .rearrange("(p x) -> p x", p=121), pdq[:])
                xt = spool.tile([WF, NMEM * WF], f32, tag="xt")
                nc.sync.dma_start(
                    xt[:],
                    pd_stage[:].rearrange("m (i k) -> k (m i)", k=WF))
                d1a = psD.tile([WL, 512], f32, tag="d1a")
                nc.tensor.matmul(d1a[:], upt32[:], xt[:, 0:512])
                d1b = psD.tile([WL, NMEM * WF - 512], f32, tag="d1b")
                nc.tensor.matmul(d1b[:], upt32[:], xt[:, 512:])
                d1s = spool.tile([WL, NMEM, WF], bf16, tag="d1s")
                d1f = d1s[:].rearrange("l m j -> l (m j)")
                nc.scalar.copy(d1f[:, 0:512], d1a[:])
                nc.scalar.copy(d1f[:, 512:], d1b[:])
                d1_stage = dpool.tile([WL, NMEM, WF], bf16, tag="d1_stage")
                nc.sync.dma_start(d1_stage[:], d1s[:])
                d1t = spool.tile([WF, NMEM, WL], bf16, tag="d1t")
                for m in range(NMEM):
                    nc.sync.dma_start(
                        d1t[:, m, :],
                        d1_stage[:, m, :].rearrange("l j -> j l"))

                s1 = spool.tile([WL, HL], f32, tag="s1")
                s2 = spool.tile([WL, HL], f32, tag="s2")
                # groups of 16 l-columns (x 30 mems = 480 psum cols)
                for g in range(6):
                    l0 = 16 * g
                    nl = min(16, WL - l0)
                    d2 = psA.tile([WL, 480], f32, tag="psa")
                    nc.tensor.matmul(
                        d2[:, 0:nl * NMEM], upt16[:],
                        d1t[:].rearrange("j m l -> j l m")[:, l0:l0 + nl, :])
                    d2v = d2[:, 0:nl * NMEM].rearrange(
                        "i (l m) -> i l m", m=NMEM)
                    nc.vector.tensor_reduce(
                        s1[:, l0:l0 + nl], d2v, axis=AX.X, op=ALU.add)
                    sqg = smpool.tile([WL, 480], f32, tag="sqg")
                    nc.scalar.square(sqg[:, 0:nl * NMEM], d2[:, 0:nl * NMEM])
                    nc.vector.tensor_reduce(
                        s2[:, l0:l0 + nl],
                        sqg[:, 0:nl * NMEM].rearrange(
                            "i (l m) -> i l m", m=NMEM),
                        axis=AX.X, op=ALU.add)

                mean = spool.tile([WL, HL], f32, tag="mean")
                nc.vector.tensor_scalar_mul(mean[:], s1[:], 1.0 / NMEM)
                ms = spool.tile([WL, HL], f32, tag="ms")
                nc.vector.tensor_tensor(out=ms[:], in0=mean[:], in1=mean[:],
                                        op=ALU.mult)
                v1 = spool.tile([WL, HL], f32, tag="v1")
                nc.vector.tensor_scalar_mul(v1[:], s2[:], 1.0 / (NMEM - 1))
                v2 = spool.tile([WL, HL], f32, tag="v2")
                nc.vector.tensor_scalar_mul(v2[:], ms[:],
                                            NMEM / (NMEM - 1.0))
                var = spool.tile([WL, HL], f32, tag="var")
                nc.vector.tensor_tensor(out=var[:], in0=v1[:], in1=v2[:],
                                        op=ALU.subtract)
                vp1 = spool.tile([WL, HL], f32, tag="vp1")
                nc.vector.tensor_scalar_add(vp1[:], var[:], 1.0)
                rv = spool.tile([WL, HL], f32, tag="rv")
                nc.vector.reciprocal(rv[:], vp1[:])
                nalpha = spool.tile([WL, 1], f32, tag="nalpha")
                nc.vector.memset(nalpha[:], -ALPHA)
                cert = spool.tile([WL, HL], f32, tag="cert")
                nc.scalar.activation(cert[:], rv[:], AF.Exp,
                                     scale=ALPHA, bias=nalpha[:])
                ts = spool.tile([WL, HL], f32, tag="ts")
                nc.sync.dma_start(ts[:], t_scores[s])
                o1 = spool.tile([WL, HL], f32, tag="o1")
                nc.vector.tensor_tensor(out=o1[:], in0=cert[:], in1=mean[:],
                                        op=ALU.mult)
                o2 = spool.tile([WL, HL], f32, tag="o2")
                nc.vector.tensor_tensor(out=o2[:], in0=o1[:], in1=ts[:],
                                        op=ALU.add)
                nc.sync.dma_start(t_out[s], o2[:])

    nc.compile()
    return nc


def _get_nc():
    if "nc" not in _CACHE:
        _CACHE["nc"] = _build()
    return _CACHE["nc"]


def _run(test_scores, train_labels, test_feat, train_feats, softmax_temp,
         trace=False):
    from concourse.bass_utils import run_bass_kernel_spmd

    test_scores = np.ascontiguousarray(test_scores, np.float32)
    train_labels = np.ascontiguousarray(train_labels, np.float32)
    test_feat = np.ascontiguousarray(test_feat, np.float32)
    train_feats = np.ascontiguousarray(train_feats, np.float32)
    temp = np.ascontiguousarray(softmax_temp, np.float32).reshape(1)

    in_maps = []
    for c in range(NCORES):
        sl = slice(SEQ_LOC * c, SEQ_LOC * (c + 1))
        in_maps.append({
            "t_scores": test_scores[0, sl],
            "t_labels": np.ascontiguousarray(train_labels[:, sl]),
            "t_tefeat": test_feat[0, sl],
            "t_trfeat": np.ascontiguousarray(train_feats[:, sl]),
            "t_temp": temp,
            "t_dnrt": np.ascontiguousarray(_DN.T),
            "t_upt32": np.ascontiguousarray(_UP.T),
            "t_upt16": _bf16(np.ascontiguousarray(_UP.T)),
        })
    nc = _get_nc()
    res = run_bass_kernel_spmd(nc, in_maps, list(range(NCORES)), trace=trace)
    out = np.empty((1, NSEQ, WL, HL), np.float32)
    for c in range(NCORES):
        out[0, SEQ_LOC * c:SEQ_LOC * (c + 1)] = res.results[c]["t_out"]
    return out, res


def _bf16(a):
    import ml_dtypes
    return a.astype(ml_dtypes.bfloat16)


def kernel(test_scores, train_labels, test_feat, train_feats, softmax_temp):
    out, _ = _run(test_scores, train_labels, test_feat, train_feats,
                  softmax_temp, trace=False)
    return out


# revision 14
# speedup vs baseline: 4867.5471x; 1.8877x over previous
"""nn_AttentionAverageStdScalingModule — Trainium2 Bass/Tile kernel.

Contract: kernel(**inputs) takes FULL unsharded inputs and returns the FULL
output (1, 16, 88, 88) f32.  The nseq axis (16) is sharded 2-per-core across
8 NeuronCores; each core runs an identical program on its 2 sequences.

Per sequence s:
  te_n = softmax_temp * test_feat[:,s]/||cols||             (256, 484)
  for each memory m: simT[j,i] = sum_c tr[c,j]*te_n[c,i]    (484j, 484i)
      ez = exp(simT * rsqrt(nsq_j))   <- per-partition scale on ScalarE
      [num;den][i] = [labels_down_m; ones]^T @ ez           (2, 484)
  pmt_down = num/den; pmt = UP @ pmt_down @ UP^T  (bilinear upsample)
  mean/unbiased-std over m, certainty = exp(A/(1+std^2)-A)
  out = certainty*mean + test_scores

Engine split: PE does sim/aggregation/norm-sums/resampling matmuls (bf16),
ScalarE does only Exp (no LUT switches), VectorE does squares/stats and a
bit-trick rsqrt (Newton x2), GpSimd does the fp32->bf16 casts.  Memory-
sums land 4-memories-per-PSUM-bank at 32-aligned partitions so copies and
DRAM relayout bounces are batched.
"""

import numpy as np

NMEM, NSEQ, C, WF, HF = 30, 16, 256, 22, 22
WL, HL = 88, 88
P2 = WF * HF            # 484
NCORES = 8
SEQ_LOC = NSEQ // NCORES  # 2
ALPHA = 20.0
JC = [128, 128, 128, 100]   # j-chunk sizes covering 484
G4 = [list(range(4 * g, min(4 * g + 4, NMEM))) for g in range(8)]
# rsqrt batches (in units of g4 groups): ramp up so exp can start early
BATCHES = [[0], [1, 2], [3, 4], [5, 6, 7]]


def _resize_matrix(n_in: int, n_out: int) -> np.ndarray:
    """Row-stochastic 1-D bilinear resize matrix (half-pixel centers,
    out-of-range taps dropped + renormalized) matching
    jax.image.resize(method='bilinear', antialias=False)."""
    M = np.zeros((n_out, n_in), np.float64)
    scale = n_in / n_out
    for i in range(n_out):
        x = (i + 0.5) * scale - 0.5
        x0 = int(np.floor(x))
        for tap, w in ((x0, 1.0 - (x - x0)), (x0 + 1, x - x0)):
            if 0 <= tap < n_in and w > 0.0:
                M[i, tap] += w
        s = M[i].sum()
        if s > 0:
            M[i] /= s
    return M.astype(np.float32)


_DN = _resize_matrix(WL, WF)   # (22, 88)  downsample
_UP = _resize_matrix(WF, WL)   # (88, 22)  upsample

_CACHE = {}


def _build():
    import concourse.bacc as bacc
    import concourse.mybir as mybir
    from concourse import tile

    f32 = mybir.dt.float32
    bf16 = mybir.dt.bfloat16
    i32 = mybir.dt.int32
    AF = mybir.ActivationFunctionType
    ALU = mybir.AluOpType
    AX = mybir.AxisListType

    nc = bacc.Bacc("TRN2", target_bir_lowering=False, debug=False,
                   num_devices=NCORES)

    t_scores = nc.dram_tensor("t_scores", [SEQ_LOC, WL, HL], f32,
                              kind="ExternalInput")
    t_labels = nc.dram_tensor("t_labels", [NMEM, SEQ_LOC, WL, HL], f32,
                              kind="ExternalInput")
    t_tefeat = nc.dram_tensor("t_tefeat", [SEQ_LOC, C, WF, HF], f32,
                              kind="ExternalInput")
    t_trfeat = nc.dram_tensor("t_trfeat", [NMEM, SEQ_LOC, C, WF, HF], f32,
                              kind="ExternalInput")
    t_temp = nc.dram_tensor("t_temp", [1], f32, kind="ExternalInput")
    t_dnrt = nc.dram_tensor("t_dnrt", [WL, WF], bf16, kind="ExternalInput")
    t_upt32 = nc.dram_tensor("t_upt32", [WF, WL], f32, kind="ExternalInput")
    t_upt16 = nc.dram_tensor("t_upt16", [WF, WL], bf16, kind="ExternalInput")
    t_ident = nc.dram_tensor("t_ident", [WL, WL], bf16, kind="ExternalInput")
    t_out = nc.dram_tensor("t_out", [SEQ_LOC, WL, HL], f32,
                           kind="ExternalOutput")

    with tile.TileContext(nc) as tc:
        with (
            tc.tile_pool(name="const", bufs=1) as cpool,
            tc.tile_pool(name="seq", bufs=1) as spool,
            tc.tile_pool(name="big", bufs=3) as bpool,
            tc.tile_pool(name="ez", bufs=4) as ezpool,
            tc.tile_pool(name="sm", bufs=2) as smpool,
            tc.tile_pool(name="dram", bufs=1, space="DRAM") as dpool,
            tc.tile_pool(name="psA", bufs=2, space="PSUM") as psA,
            tc.tile_pool(name="psB", bufs=2, space="PSUM") as psB,
            tc.tile_pool(name="psC", bufs=2, space="PSUM") as psC,
            tc.tile_pool(name="psD", bufs=1, space="PSUM") as psD,
        ):
            # ---- constants ----
            ones_col32 = cpool.tile([128, 1], f32)
            nc.vector.memset(ones_col32[:], 1.0)
            ones_col16 = cpool.tile([128, 1], bf16)
            nc.vector.memset(ones_col16[:], 1.0)
            ones_row32 = cpool.tile([1, 128], f32)
            nc.vector.memset(ones_row32[:], 1.0)
            dnrt = cpool.tile([WL, WF], bf16)      # DN^T (88, 22)
            nc.sync.dma_start(dnrt[:], t_dnrt[:])
            upt32 = cpool.tile([WF, WL], f32)      # UP^T (22, 88)
            nc.sync.dma_start(upt32[:], t_upt32[:])
            upt16 = cpool.tile([WF, WL], bf16)
            nc.sync.dma_start(upt16[:], t_upt16[:])
            ident = cpool.tile([WL, WL], bf16)
            nc.sync.dma_start(ident[:], t_ident[:])
            temp_t = cpool.tile([1, 1], f32)
            nc.sync.dma_start(temp_t[:], t_temp[:])

            for s in range(SEQ_LOC):
                # =========== test-feature prep ===========
                te32 = spool.tile([128, 2, P2], f32, tag="te32")
                nc.sync.dma_start(
                    te32[:],
                    t_tefeat[s].rearrange("(h p) w hh -> p h (w hh)", p=128))
                sq_te = spool.tile([128, 2, P2], f32, tag="sq_te")
                nc.vector.tensor_tensor(
                    out=sq_te[:], in0=te32[:], in1=te32[:], op=ALU.mult)
                nsq_te = psA.tile([1, P2], f32, tag="psa")
                for h in range(2):
                    nc.tensor.matmul(nsq_te[:], ones_col32[:], sq_te[:, h, :],
                                     start=(h == 0), stop=(h == 1))
                nrm_te = spool.tile([1, P2], f32, tag="nrm_te")
                nc.scalar.sqrt(nrm_te[:], nsq_te[:])
                inv_te = spool.tile([1, P2], f32, tag="inv_te")
                nc.vector.reciprocal(inv_te[:], nrm_te[:])
                tsc = spool.tile([1, P2], f32, tag="tsc")
                nc.vector.tensor_scalar_mul(tsc[:], inv_te[:],
                                            temp_t[0:1, 0:1])
                tsc_b = psA.tile([128, P2], f32, tag="psa")
                nc.tensor.matmul(tsc_b[:], ones_row32[:], tsc[:])
                te_n = spool.tile([128, 2, P2], bf16, tag="te_n")
                for h in range(2):
                    nc.vector.tensor_tensor(
                        out=te_n[:, h, :], in0=te32[:, h, :], in1=tsc_b[:],
                        op=ALU.mult)

                # =========== labels: bilinear 88->22, flatten j-major ======
                labs32 = spool.tile([WL, NMEM, HL], f32, tag="labs32")
                nc.sync.dma_start(
                    labs32[:], t_labels[:, s].rearrange("m w hh -> w m hh"))
                labs = spool.tile([WL, NMEM, HL], bf16, tag="labs")
                nc.gpsimd.tensor_copy(labs[:], labs32[:])
                ld_stage = dpool.tile([NMEM, P2], bf16, tag="ld_stage")
                ldsb = spool.tile([WF, NMEM, WF], bf16, tag="ldsb")
                for m in range(NMEM):
                    b1 = psA.tile([WL, WF], f32, tag="psa")
                    nc.tensor.matmul(b1[:], labs[:, m, :], dnrt[:])
                    b1s = smpool.tile([WL, WF], bf16, tag="b1s")
                    nc.vector.tensor_copy(b1s[:], b1[:])
                    ldp = psA.tile([WF, WF], f32, tag="psa")
                    nc.tensor.matmul(ldp[:], b1s[:], dnrt[:])
                    nc.vector.tensor_copy(ldsb[:, m, :], ldp[:])
                nc.sync.dma_start(
                    ld_stage[:].rearrange("m (i k) -> i m k", k=WF), ldsb[:])
                ldj = spool.tile([128, NMEM, 4, 2], bf16, tag="ldj")
                nc.vector.memset(ldj[:], 0.0)
                nc.vector.memset(ldj[:, :, 0:3, 1], 1.0)
                nc.vector.memset(ldj[0:100, :, 3, 1], 1.0)
                for q in range(4):
                    pq = JC[q]
                    nc.sync.dma_start(
                        ldj[0:pq, :, q, 0],
                        ld_stage[:, 128 * q:128 * q + pq]
                        .rearrange("m p -> p m"))

                nrm_stage = dpool.tile([NMEM, 512], f32, tag="nrm_stage")
                nd_stage = dpool.tile([2, NMEM, P2], f32, tag="nd_stage")
                invj = spool.tile([128, NMEM, 4], f32, tag="invj")

                trbf_t = {}
                # =========== main per-memory pipeline, in rsqrt batches ====
                for batch in BATCHES:
                    # -- phase 1: load + cast + squares + norm-sums --
                    for g in batch:
                        nsqp = psC.tile([128, P2], f32, tag="psc")
                        for m in G4[g]:
                            r = 32 * (m % 4)
                            tr32 = bpool.tile([128, 2, P2], f32, tag="tr32", bufs=4)
                            nc.sync.dma_start(
                                tr32[:],
                                t_trfeat[m, s]
                                .rearrange("(h p) w hh -> p h (w hh)", p=128))
                            trbf = bpool.tile([128, 2, P2], bf16, tag="trbf", bufs=20)
                            nc.gpsimd.tensor_copy(trbf[:], tr32[:])
                            trbf_t[m] = trbf
                            sqbf = bpool.tile([128, 2, P2], bf16, tag="sqbf")
                            nc.vector.tensor_tensor(
                                out=sqbf[:], in0=trbf[:], in1=trbf[:],
                                op=ALU.mult)
                            for h in range(2):
                                nc.tensor.matmul(
                                    nsqp[r:r + 1, :], ones_col16[:],
                                    sqbf[:, h, :],
                                    start=(h == 0), stop=(h == 1),
                                    tile_position=(0, r))
                        nsqsb = smpool.tile([128, P2], f32, tag="nsqsb")
                        nc.vector.tensor_copy(nsqsb[:], nsqp[:])
                        m0 = G4[g][0]
                        nmg = len(G4[g])
                        nc.sync.dma_start(
                            nrm_stage[m0:m0 + nmg, 0:P2],
                            nsqsb[0:(nmg - 1) * 32 + 1:32, :])
                    # -- rsqrt for the whole batch (bit trick + 2 Newton) --
                    mlo = G4[batch[0]][0]
                    mhi = G4[batch[-1]][-1] + 1
                    nc.sync.dma_start(
                        invj[:, mlo:mhi, :],
                        nrm_stage[mlo:mhi].rearrange("m (q p) -> p m q",
                                                     p=128))
                    xv = invj[:, mlo:mhi, :]
                    xh = smpool.tile([128, NMEM, 4], f32, tag="xh")
                    nc.vector.tensor_scalar_mul(xh[:, mlo:mhi, :], xv, 0.5)
                    yv = smpool.tile([128, NMEM, 4], f32, tag="yv")
                    nc.vector.tensor_scalar(
                        out=yv[:, mlo:mhi, :].bitcast(i32),
                        in0=xv.bitcast(i32),
                        scalar1=1, scalar2=None,
                        op0=ALU.logical_shift_right)
                    nc.vector.tensor_scalar(
                        out=yv[:, mlo:mhi, :].bitcast(i32),
                        in0=yv[:, mlo:mhi, :].bitcast(i32),
                        scalar1=-1, scalar2=0x5F3759DF,
                        op0=ALU.mult, op1=ALU.add)
                    tv = smpool.tile([128, NMEM, 4], f32, tag="tv")
                    for _ in range(2):
                        nc.vector.tensor_tensor(
                            out=tv[:, mlo:mhi, :], in0=yv[:, mlo:mhi, :],
                            in1=yv[:, mlo:mhi, :], op=ALU.mult)
                        nc.vector.tensor_tensor(
                            out=tv[:, mlo:mhi, :], in0=tv[:, mlo:mhi, :],
                            in1=xh[:, mlo:mhi, :], op=ALU.mult)
                        nc.vector.tensor_scalar(
                            out=tv[:, mlo:mhi, :], in0=tv[:, mlo:mhi, :],
                            scalar1=-1.0, scalar2=1.5,
                            op0=ALU.mult, op1=ALU.add)
                        nc.vector.tensor_tensor(
                            out=yv[:, mlo:mhi, :], in0=yv[:, mlo:mhi, :],
                            in1=tv[:, mlo:mhi, :], op=ALU.mult)
                    nc.vector.tensor_copy(xv, yv[:, mlo:mhi, :])

                    # -- phase 2: sim + exp + aggregate --
                    for g in batch:
                        agp = psB.tile([128, P2], f32, tag="psb")
                        for m in G4[g]:
                            r = 32 * (m % 4)
                            trbf = trbf_t.pop(m)
                            for q in range(4):
                                pq = JC[q]
                                j0 = 128 * q
                                st = psA.tile([128, P2], f32, tag="psa")
                                for h in range(2):
                                    nc.tensor.matmul(
                                        st[0:pq, :],
                                        trbf[:, h, j0:j0 + pq],
                                        te_n[:, h, :],
                                        start=(h == 0), stop=(h == 1))
                                ez = ezpool.tile([128, P2], bf16, tag="ez")
                                nc.scalar.activation(
                                    ez[0:pq, :], st[0:pq, :], AF.Exp,
                                    scale=invj[0:pq, m, q:q + 1])
                                nc.tensor.matmul(
                                    agp[r:r + 2, :], ldj[0:pq, m, q, :],
                                    ez[0:pq, :],
                                    start=(q == 0), stop=(q == 3),
                                    tile_position=(0, r))
                        ndsb = smpool.tile([128, P2], f32, tag="ndsb")
                        nc.vector.tensor_copy(ndsb[:], agp[:])
                        m0 = G4[g][0]
                        nmg = len(G4[g])
                        nc.sync.dma_start(
                            nd_stage[0, m0:m0 + nmg, :],
                            ndsb[0:(nmg - 1) * 32 + 1:32, :])
                        nc.sync.dma_start(
                            nd_stage[1, m0:m0 + nmg, :],
                            ndsb[1:(nmg - 1) * 32 + 2:32, :])

                # =========== batched num/den division ===========
                numt = spool.tile([121, 120], f32, tag="numt")
                nc.sync.dma_start(
                    numt[:], nd_stage[0].rearrange("m j -> (m j)")
                    .rearrange("(p x) -> p x", p=121))
                dent = spool.tile([121, 120], f32, tag="dent")
                nc.sync.dma_start(
                    dent[:], nd_stage[1].rearrange("m j -> (m j)")
                    .rearrange("(p x) -> p x", p=121))
                rden = spool.tile([121, 120], f32, tag="rden")
                nc.vector.reciprocal(rden[:], dent[:])
                pdq = spool.tile([121, 120], f32, tag="pdq")
                nc.vector.tensor_tensor(out=pdq[:], in0=numt[:], in1=rden[:],
                                        op=ALU.mult)
                pd_stage = dpool.tile([NMEM, P2], f32, tag="pd_stage")
                nc.sync.dma_start(
                    pd_stage[:].rearrange("m j -> (m j)")
                    .rearrange("(p x) -> p x", p=121), pdq[:])

                # =========== upsample + stats + output ===========
                xt = spool.tile([WF, NMEM * WF], f32, tag="xt")
                nc.sync.dma_start(
                    xt[:],
                    pd_stage[:].rearrange("m (i k) -> k (m i)", k=WF))
                d1a = psD.tile([WL, 512], f32, tag="d1a")
                nc.tensor.matmul(d1a[:], upt32[:], xt[:, 0:512])
                d1b = psD.tile([WL, NMEM * WF - 512], f32, tag="d1b")
                nc.tensor.matmul(d1b[:], upt32[:], xt[:, 512:])
                d1s = spool.tile([WL, NMEM, WF], bf16, tag="d1s")
                d1f = d1s[:].rearrange("l m j -> l (m j)")
                nc.vector.tensor_copy(d1f[:, 0:512], d1a[:])
                nc.vector.tensor_copy(d1f[:, 512:], d1b[:])
                d1t = spool.tile([WF, NMEM, WL], bf16, tag="d1t")
                for m in range(NMEM):
                    trp = psD.tile([WF, WL], bf16,
                                   tag=("d1a" if m % 2 else "d1b"))
                    nc.tensor.transpose(trp[:], d1s[:, m, :], ident[:])
                    nc.vector.tensor_copy(d1t[:, m, :], trp[:])

                s1 = spool.tile([WL, HL], f32, tag="s1")
                s2 = spool.tile([WL, HL], f32, tag="s2")
                for gg in range(6):
                    l0 = 16 * gg
                    nl = min(16, WL - l0)
                    d2 = psA.tile([WL, 480], f32, tag="psa")
                    nc.tensor.matmul(
                        d2[:, 0:nl * NMEM], upt16[:],
                        d1t[:].rearrange("j m l -> j l m")[:, l0:l0 + nl, :])
                    d2v = d2[:, 0:nl * NMEM].rearrange(
                        "i (l m) -> i l m", m=NMEM)
                    nc.vector.tensor_reduce(
                        s1[:, l0:l0 + nl], d2v, axis=AX.X, op=ALU.add)
                    sqg = smpool.tile([WL, 480], f32, tag="sqg")
                    nc.scalar.square(sqg[:, 0:nl * NMEM], d2[:, 0:nl * NMEM])
                    nc.vector.tensor_reduce(
                        s2[:, l0:l0 + nl],
                        sqg[:, 0:nl * NMEM].rearrange(
                            "i (l m) -> i l m", m=NMEM),
                        axis=AX.X, op=ALU.add)

                mean = spool.tile([WL, HL], f32, tag="mean")
                nc.vector.tensor_scalar_mul(mean[:], s1[:], 1.0 / NMEM)
                ms = spool.tile([WL, HL], f32, tag="ms")
                nc.vector.tensor_tensor(out=ms[:], in0=mean[:], in1=mean[:],
                                        op=ALU.mult)
                v1 = spool.tile([WL, HL], f32, tag="v1")
                nc.vector.tensor_scalar_mul(v1[:], s2[:], 1.0 / (NMEM - 1))
                v2 = spool.tile([WL, HL], f32, tag="v2")
                nc.vector.tensor_scalar_mul(v2[:], ms[:],
                                            NMEM / (NMEM - 1.0))
                var = spool.tile([WL, HL], f32, tag="var")
                nc.vector.tensor_tensor(out=var[:], in0=v1[:], in1=v2[:],
                                        op=ALU.subtract)
                vp1 = spool.tile([WL, HL], f32, tag="vp1")
                nc.vector.tensor_scalar_add(vp1[:], var[:], 1.0)
                rv = spool.tile([WL, HL], f32, tag="rv")
                nc.vector.reciprocal(rv[:], vp1[:])
                nalpha = spool.tile([WL, 1], f32, tag="nalpha")
                nc.vector.memset(nalpha[:], -ALPHA)
                cert = spool.tile([WL, HL], f32, tag="cert")
                nc.scalar.activation(cert[:], rv[:], AF.Exp,
                                     scale=ALPHA, bias=nalpha[:])
                ts = spool.tile([WL, HL], f32, tag="ts")
                nc.sync.dma_start(ts[:], t_scores[s])
                o1 = spool.tile([WL, HL], f32, tag="o1")
                nc.vector.tensor_tensor(out=o1[:], in0=cert[:], in1=mean[:],
                                        op=ALU.mult)
                o2 = spool.tile([WL, HL], f32, tag="o2")
                nc.vector.tensor_tensor(out=o2[:], in0=o1[:], in1=ts[:],
                                        op=ALU.add)
                nc.sync.dma_start(t_out[s], o2[:])

    nc.compile()
    return nc


def _get_nc():
    if "nc" not in _CACHE:
        _CACHE["nc"] = _build()
    return _CACHE["nc"]


def _bf16(a):
    import ml_dtypes
    return np.ascontiguousarray(a).astype(ml_dtypes.bfloat16)


def _run(test_scores, train_labels, test_feat, train_feats, softmax_temp,
         trace=False):
    from concourse.bass_utils import run_bass_kernel_spmd

    test_scores = np.ascontiguousarray(test_scores, np.float32)
    train_labels = np.ascontiguousarray(train_labels, np.float32)
    test_feat = np.ascontiguousarray(test_feat, np.float32)
    train_feats = np.ascontiguousarray(train_feats, np.float32)
    temp = np.ascontiguousarray(softmax_temp, np.float32).reshape(1)

    in_maps = []
    for c in range(NCORES):
        sl = slice(SEQ_LOC * c, SEQ_LOC * (c + 1))
        in_maps.append({
            "t_scores": test_scores[0, sl],
            "t_labels": np.ascontiguousarray(train_labels[:, sl]),
            "t_tefeat": test_feat[0, sl],
            "t_trfeat": np.ascontiguousarray(train_feats[:, sl]),
            "t_temp": temp,
            "t_dnrt": _bf16(_DN.T),
            "t_upt32": np.ascontiguousarray(_UP.T),
            "t_upt16": _bf16(_UP.T),
            "t_ident": _bf16(np.eye(WL, dtype=np.float32)),
        })
    nc = _get_nc()
    res = run_bass_kernel_spmd(nc, in_maps, list(range(NCORES)), trace=trace)
    out = np.empty((1, NSEQ, WL, HL), np.float32)
    for c in range(NCORES):
        out[0, SEQ_LOC * c:SEQ_LOC * (c + 1)] = res.results[c]["t_out"]
    return out, res


def kernel(test_scores, train_labels, test_feat, train_feats, softmax_temp):
    out, _ = _run(test_scores, train_labels, test_feat, train_feats,
                  softmax_temp, trace=False)
    return out


# revision 15
# speedup vs baseline: 4962.1138x; 1.0194x over previous
"""nn_AttentionAverageStdScalingModule — Trainium2 Bass/Tile kernel.

Contract: kernel(**inputs) takes FULL unsharded inputs and returns the FULL
output (1, 16, 88, 88) f32.  The nseq axis (16) is sharded 2-per-core across
8 NeuronCores; each core runs an identical program on its 2 sequences.

Per sequence s:
  te_n = softmax_temp * test_feat[:,s]/||cols||             (256, 484)
  for each memory m: simT[j,i] = sum_c tr[c,j]*te_n[c,i]    (484j, 484i)
      ez = exp(simT * rsqrt(nsq_j))   <- per-partition scale on ScalarE
      [num;den][i] = [labels_down_m; ones]^T @ ez           (2, 484)
  pmt_down = num/den; pmt = UP @ pmt_down @ UP^T  (bilinear upsample)
  mean/unbiased-std over m, certainty = exp(A/(1+std^2)-A)
  out = certainty*mean + test_scores

Engine split: PE does sim/aggregation/norm-sums/resampling matmuls (bf16),
ScalarE does only Exp (no LUT switches), VectorE does squares/stats and a
bit-trick rsqrt (Newton x2), GpSimd does the fp32->bf16 casts.  Memory-
sums land 4-memories-per-PSUM-bank at 32-aligned partitions so copies and
DRAM relayout bounces are batched.
"""

import numpy as np

NMEM, NSEQ, C, WF, HF = 30, 16, 256, 22, 22
WL, HL = 88, 88
P2 = WF * HF            # 484
NCORES = 8
SEQ_LOC = NSEQ // NCORES  # 2
ALPHA = 20.0
JC = [128, 128, 128, 100]   # j-chunk sizes covering 484
G4 = [list(range(4 * g, min(4 * g + 4, NMEM))) for g in range(8)]
# rsqrt batches (in units of g4 groups): ramp up so exp can start early
BATCHES = [[0], [1, 2], [3, 4], [5, 6, 7]]


def _resize_matrix(n_in: int, n_out: int) -> np.ndarray:
    """Row-stochastic 1-D bilinear resize matrix (half-pixel centers,
    out-of-range taps dropped + renormalized) matching
    jax.image.resize(method='bilinear', antialias=False)."""
    M = np.zeros((n_out, n_in), np.float64)
    scale = n_in / n_out
    for i in range(n_out):
        x = (i + 0.5) * scale - 0.5
        x0 = int(np.floor(x))
        for tap, w in ((x0, 1.0 - (x - x0)), (x0 + 1, x - x0)):
            if 0 <= tap < n_in and w > 0.0:
                M[i, tap] += w
        s = M[i].sum()
        if s > 0:
            M[i] /= s
    return M.astype(np.float32)


_DN = _resize_matrix(WL, WF)   # (22, 88)  downsample
_UP = _resize_matrix(WF, WL)   # (88, 22)  upsample

_CACHE = {}


def _build():
    import concourse.bacc as bacc
    import concourse.mybir as mybir
    from concourse import tile

    f32 = mybir.dt.float32
    bf16 = mybir.dt.bfloat16
    i32 = mybir.dt.int32
    AF = mybir.ActivationFunctionType
    ALU = mybir.AluOpType
    AX = mybir.AxisListType

    nc = bacc.Bacc("TRN2", target_bir_lowering=False, debug=False,
                   num_devices=NCORES)

    t_scores = nc.dram_tensor("t_scores", [SEQ_LOC, WL, HL], f32,
                              kind="ExternalInput")
    t_labels = nc.dram_tensor("t_labels", [NMEM, SEQ_LOC, WL, HL], f32,
                              kind="ExternalInput")
    t_tefeat = nc.dram_tensor("t_tefeat", [SEQ_LOC, C, WF, HF], f32,
                              kind="ExternalInput")
    t_trfeat = nc.dram_tensor("t_trfeat", [NMEM, SEQ_LOC, C, WF, HF], f32,
                              kind="ExternalInput")
    t_temp = nc.dram_tensor("t_temp", [1], f32, kind="ExternalInput")
    t_dnrt = nc.dram_tensor("t_dnrt", [WL, WF], bf16, kind="ExternalInput")
    t_upt32 = nc.dram_tensor("t_upt32", [WF, WL], f32, kind="ExternalInput")
    t_upt16 = nc.dram_tensor("t_upt16", [WF, WL], bf16, kind="ExternalInput")
    t_ident = nc.dram_tensor("t_ident", [WL, WL], bf16, kind="ExternalInput")
    t_out = nc.dram_tensor("t_out", [SEQ_LOC, WL, HL], f32,
                           kind="ExternalOutput")

    with tile.TileContext(nc) as tc:
        with (
            tc.tile_pool(name="const", bufs=1) as cpool,
            tc.tile_pool(name="seq", bufs=1) as spool,
            tc.tile_pool(name="big", bufs=3) as bpool,
            tc.tile_pool(name="ez", bufs=4) as ezpool,
            tc.tile_pool(name="sm", bufs=2) as smpool,
            tc.tile_pool(name="dram", bufs=1, space="DRAM") as dpool,
            tc.tile_pool(name="psA", bufs=2, space="PSUM") as psA,
            tc.tile_pool(name="psB", bufs=2, space="PSUM") as psB,
            tc.tile_pool(name="psC", bufs=2, space="PSUM") as psC,
            tc.tile_pool(name="psD", bufs=1, space="PSUM") as psD,
        ):
            # ---- constants ----
            ones_col32 = cpool.tile([128, 1], f32)
            nc.vector.memset(ones_col32[:], 1.0)
            ones_col16 = cpool.tile([128, 1], bf16)
            nc.vector.memset(ones_col16[:], 1.0)
            ones_row32 = cpool.tile([1, 128], f32)
            nc.vector.memset(ones_row32[:], 1.0)
            dnrt = cpool.tile([WL, WF], bf16)      # DN^T (88, 22)
            nc.sync.dma_start(dnrt[:], t_dnrt[:])
            upt32 = cpool.tile([WF, WL], f32)      # UP^T (22, 88)
            nc.sync.dma_start(upt32[:], t_upt32[:])
            upt16 = cpool.tile([WF, WL], bf16)
            nc.sync.dma_start(upt16[:], t_upt16[:])
            ident = cpool.tile([WL, WL], bf16)
            nc.sync.dma_start(ident[:], t_ident[:])
            temp_t = cpool.tile([1, 1], f32)
            nc.sync.dma_start(temp_t[:], t_temp[:])

            for s in range(SEQ_LOC):
                # =========== test-feature prep ===========
                te32 = spool.tile([128, 2, P2], f32, tag="te32")
                nc.sync.dma_start(
                    te32[:],
                    t_tefeat[s].rearrange("(h p) w hh -> p h (w hh)", p=128))
                sq_te = spool.tile([128, 2, P2], f32, tag="sq_te")
                nc.vector.tensor_tensor(
                    out=sq_te[:], in0=te32[:], in1=te32[:], op=ALU.mult)
                nsq_te = psA.tile([1, P2], f32, tag="psa")
                for h in range(2):
                    nc.tensor.matmul(nsq_te[:], ones_col32[:], sq_te[:, h, :],
                                     start=(h == 0), stop=(h == 1))
                nrm_te = spool.tile([1, P2], f32, tag="nrm_te")
                nc.scalar.sqrt(nrm_te[:], nsq_te[:])
                inv_te = spool.tile([1, P2], f32, tag="inv_te")
                nc.vector.reciprocal(inv_te[:], nrm_te[:])
                tsc = spool.tile([1, P2], f32, tag="tsc")
                nc.vector.tensor_scalar_mul(tsc[:], inv_te[:],
                                            temp_t[0:1, 0:1])
                tsc_b = psA.tile([128, P2], f32, tag="psa")
                nc.tensor.matmul(tsc_b[:], ones_row32[:], tsc[:])
                te_n = spool.tile([128, 2, P2], bf16, tag="te_n")
                for h in range(2):
                    nc.vector.tensor_tensor(
                        out=te_n[:, h, :], in0=te32[:, h, :], in1=tsc_b[:],
                        op=ALU.mult)

                # =========== labels: bilinear 88->22, flatten j-major ======
                labs = spool.tile([WL, NMEM, HL], bf16, tag="labs")
                nc.gpsimd.dma_start(
                    labs[:], t_labels[:, s].rearrange("m w hh -> w m hh"))
                ld_stage = dpool.tile([NMEM, P2], bf16, tag="ld_stage")
                ldsb = spool.tile([WF, NMEM, WF], bf16, tag="ldsb")
                for m in range(NMEM):
                    b1 = psA.tile([WL, WF], f32, tag="psa")
                    nc.tensor.matmul(b1[:], labs[:, m, :], dnrt[:])
                    b1s = smpool.tile([WL, WF], bf16, tag="b1s")
                    nc.vector.tensor_copy(b1s[:], b1[:])
                    ldp = psA.tile([WF, WF], f32, tag="psa")
                    nc.tensor.matmul(ldp[:], b1s[:], dnrt[:])
                    nc.vector.tensor_copy(ldsb[:, m, :], ldp[:])
                nc.sync.dma_start(
                    ld_stage[:].rearrange("m (i k) -> i m k", k=WF), ldsb[:])
                ldj = spool.tile([128, NMEM, 4, 2], bf16, tag="ldj")
                nc.vector.memset(ldj[:], 0.0)
                nc.vector.memset(ldj[:, :, 0:3, 1], 1.0)
                nc.vector.memset(ldj[0:100, :, 3, 1], 1.0)
                for q in range(4):
                    pq = JC[q]
                    nc.sync.dma_start(
                        ldj[0:pq, :, q, 0],
                        ld_stage[:, 128 * q:128 * q + pq]
                        .rearrange("m p -> p m"))

                nrm_stage = dpool.tile([NMEM, 512], f32, tag="nrm_stage")
                nd_stage = dpool.tile([2, NMEM, P2], f32, tag="nd_stage")
                invj = spool.tile([128, NMEM, 4], f32, tag="invj")

                trbf_t = {}
                # =========== main per-memory pipeline, in rsqrt batches ====
                for batch in BATCHES:
                    # -- phase 1: load + cast + squares + norm-sums --
                    for g in batch:
                        nsqp = psC.tile([128, P2], f32, tag="psc")
                        gm = G4[g]
                        pairs = [gm[i:i + 2] for i in range(0, len(gm), 2)]
                        sq_t = {}
                        for pr in pairs:
                            trp2 = bpool.tile([128, 2, 2, P2], bf16,
                                              tag="trbf", bufs=10)
                            for k, m in enumerate(pr):
                                nc.gpsimd.dma_start(
                                    trp2[:, k, :, :],
                                    t_trfeat[m, s]
                                    .rearrange("(h p) w hh -> p h (w hh)",
                                               p=128))
                                trbf_t[m] = trp2[:, k, :, :]
                            sq2 = bpool.tile([128, 2, 2, P2], bf16,
                                             tag="sqbf")
                            nc.vector.tensor_tensor(
                                out=sq2[:], in0=trp2[:], in1=trp2[:],
                                op=ALU.mult)
                            for k, m in enumerate(pr):
                                sq_t[m] = sq2[:, k, :, :]
                        for h in range(2):
                            for m in gm:
                                r = 32 * (m % 4)
                                nc.tensor.matmul(
                                    nsqp[r:r + 1, :], ones_col16[:],
                                    sq_t[m][:, h, :],
                                    start=(h == 0), stop=(h == 1),
                                    tile_position=(0, r))
                        nsqsb = smpool.tile([128, P2], f32, tag="nsqsb")
                        nc.vector.tensor_copy(nsqsb[:], nsqp[:])
                        m0 = G4[g][0]
                        nmg = len(G4[g])
                        nc.sync.dma_start(
                            nrm_stage[m0:m0 + nmg, 0:P2],
                            nsqsb[0:(nmg - 1) * 32 + 1:32, :])
                    # -- rsqrt for the whole batch (bit trick + 2 Newton) --
                    mlo = G4[batch[0]][0]
                    mhi = G4[batch[-1]][-1] + 1
                    nc.sync.dma_start(
                        invj[:, mlo:mhi, :],
                        nrm_stage[mlo:mhi].rearrange("m (q p) -> p m q",
                                                     p=128))
                    xv = invj[:, mlo:mhi, :]
                    xh = smpool.tile([128, NMEM, 4], f32, tag="xh")
                    nc.vector.tensor_scalar_mul(xh[:, mlo:mhi, :], xv, 0.5)
                    yv = smpool.tile([128, NMEM, 4], f32, tag="yv")
                    nc.vector.tensor_scalar(
                        out=yv[:, mlo:mhi, :].bitcast(i32),
                        in0=xv.bitcast(i32),
                        scalar1=1, scalar2=None,
                        op0=ALU.logical_shift_right)
                    nc.vector.tensor_scalar(
                        out=yv[:, mlo:mhi, :].bitcast(i32),
                        in0=yv[:, mlo:mhi, :].bitcast(i32),
                        scalar1=-1, scalar2=0x5F3759DF,
                        op0=ALU.mult, op1=ALU.add)
                    tv = smpool.tile([128, NMEM, 4], f32, tag="tv")
                    for _ in range(2):
                        nc.vector.tensor_tensor(
                            out=tv[:, mlo:mhi, :], in0=yv[:, mlo:mhi, :],
                            in1=yv[:, mlo:mhi, :], op=ALU.mult)
                        nc.vector.tensor_tensor(
                            out=tv[:, mlo:mhi, :], in0=tv[:, mlo:mhi, :],
                            in1=xh[:, mlo:mhi, :], op=ALU.mult)
                        nc.vector.tensor_scalar(
                            out=tv[:, mlo:mhi, :], in0=tv[:, mlo:mhi, :],
                            scalar1=-1.0, scalar2=1.5,
                            op0=ALU.mult, op1=ALU.add)
                        nc.vector.tensor_tensor(
                            out=yv[:, mlo:mhi, :], in0=yv[:, mlo:mhi, :],
                            in1=tv[:, mlo:mhi, :], op=ALU.mult)
                    nc.vector.tensor_copy(xv, yv[:, mlo:mhi, :])

                    # -- phase 2: sim + exp + aggregate --
                    for g in batch:
                        agp = psB.tile([128, P2], f32, tag="psb")
                        gm = G4[g]
                        for q in range(4):
                            pq = JC[q]
                            j0 = 128 * q
                            ez_t = {}
                            for m in gm:
                                trbf = trbf_t[m]
                                st = psA.tile([128, P2], f32, tag="psa")
                                for h in range(2):
                                    nc.tensor.matmul(
                                        st[0:pq, :],
                                        trbf[:, h, j0:j0 + pq],
                                        te_n[:, h, :],
                                        start=(h == 0), stop=(h == 1))
                                ez = ezpool.tile([128, P2], bf16, tag="ez",
                                                 bufs=6)
                                nc.scalar.activation(
                                    ez[0:pq, :], st[0:pq, :], AF.Exp,
                                    scale=invj[0:pq, m, q:q + 1])
                                ez_t[m] = ez
                            for m in gm:
                                r = 32 * (m % 4)
                                nc.tensor.matmul(
                                    agp[r:r + 2, :], ldj[0:pq, m, q, :],
                                    ez_t[m][0:pq, :],
                                    start=(q == 0), stop=(q == 3),
                                    tile_position=(0, r))
                        for m in gm:
                            trbf_t.pop(m, None)
                        ndsb = smpool.tile([128, P2], f32, tag="ndsb")
                        nc.vector.tensor_copy(ndsb[:], agp[:])
                        m0 = G4[g][0]
                        nmg = len(G4[g])
                        nc.sync.dma_start(
                            nd_stage[0, m0:m0 + nmg, :],
                            ndsb[0:(nmg - 1) * 32 + 1:32, :])
                        nc.sync.dma_start(
                            nd_stage[1, m0:m0 + nmg, :],
                            ndsb[1:(nmg - 1) * 32 + 2:32, :])

                # =========== batched num/den division ===========
                numt = spool.tile([121, 120], f32, tag="numt")
                nc.sync.dma_start(
                    numt[:], nd_stage[0].rearrange("m j -> (m j)")
                    .rearrange("(p x) -> p x", p=121))
                dent = spool.tile([121, 120], f32, tag="dent")
                nc.sync.dma_start(
                    dent[:], nd_stage[1].rearrange("m j -> (m j)")
                    .rearrange("(p x) -> p x", p=121))
                rden = spool.tile([121, 120], f32, tag="rden")
                nc.vector.reciprocal(rden[:], dent[:])
                pdq = spool.tile([121, 120], f32, tag="pdq")
                nc.vector.tensor_tensor(out=pdq[:], in0=numt[:], in1=rden[:],
                                        op=ALU.mult)
                pd_stage = dpool.tile([NMEM, P2], f32, tag="pd_stage")
                nc.sync.dma_start(
                    pd_stage[:].rearrange("m j -> (m j)")
                    .rearrange("(p x) -> p x", p=121), pdq[:])

                # =========== upsample + stats + output ===========
                xt = spool.tile([WF, NMEM * WF], f32, tag="xt")
                nc.sync.dma_start(
                    xt[:],
                    pd_stage[:].rearrange("m (i k) -> k (m i)", k=WF))
                d1a = psD.tile([WL, 512], f32, tag="d1a")
                nc.tensor.matmul(d1a[:], upt32[:], xt[:, 0:512])
                d1b = psD.tile([WL, NMEM * WF - 512], f32, tag="d1b")
                nc.tensor.matmul(d1b[:], upt32[:], xt[:, 512:])
                d1s = spool.tile([WL, NMEM, WF], bf16, tag="d1s")
                d1f = d1s[:].rearrange("l m j -> l (m j)")
                nc.vector.tensor_copy(d1f[:, 0:512], d1a[:])
                nc.vector.tensor_copy(d1f[:, 512:], d1b[:])
                d1t = spool.tile([WF, NMEM, WL], bf16, tag="d1t")
                for m in range(NMEM):
                    trp = psD.tile([WF, WL], bf16,
                                   tag=("d1a" if m % 2 else "d1b"))
                    nc.tensor.transpose(trp[:], d1s[:, m, :], ident[:])
                    nc.vector.tensor_copy(d1t[:, m, :], trp[:])

                s1 = spool.tile([WL, HL], f32, tag="s1")
                s2 = spool.tile([WL, HL], f32, tag="s2")
                for gg in range(6):
                    l0 = 16 * gg
                    nl = min(16, WL - l0)
                    d2 = psA.tile([WL, 480], f32, tag="psa")
                    nc.tensor.matmul(
                        d2[:, 0:nl * NMEM], upt16[:],
                        d1t[:].rearrange("j m l -> j l m")[:, l0:l0 + nl, :])
                    d2v = d2[:, 0:nl * NMEM].rearrange(
                        "i (l m) -> i l m", m=NMEM)
                    nc.vector.tensor_reduce(
                        s1[:, l0:l0 + nl], d2v, axis=AX.X, op=ALU.add)
                    sqg = smpool.tile([WL, 480], f32, tag="sqg")
                    nc.scalar.square(sqg[:, 0:nl * NMEM], d2[:, 0:nl * NMEM])
                    nc.vector.tensor_reduce(
                        s2[:, l0:l0 + nl],
                        sqg[:, 0:nl * NMEM].rearrange(
                            "i (l m) -> i l m", m=NMEM),
                        axis=AX.X, op=ALU.add)

                mean = spool.tile([WL, HL], f32, tag="mean")
                nc.vector.tensor_scalar_mul(mean[:], s1[:], 1.0 / NMEM)
                ms = spool.tile([WL, HL], f32, tag="ms")
                nc.vector.tensor_tensor(out=ms[:], in0=mean[:], in1=mean[:],
                                        op=ALU.mult)
                v1 = spool.tile([WL, HL], f32, tag="v1")
                nc.vector.tensor_scalar_mul(v1[:], s2[:], 1.0 / (NMEM - 1))
                v2 = spool.tile([WL, HL], f32, tag="v2")
                nc.vector.tensor_scalar_mul(v2[:], ms[:],
                                            NMEM / (NMEM - 1.0))
                var = spool.tile([WL, HL], f32, tag="var")
                nc.vector.tensor_tensor(out=var[:], in0=v1[:], in1=v2[:],
                                        op=ALU.subtract)
                vp1 = spool.tile([WL, HL], f32, tag="vp1")
                nc.vector.tensor_scalar_add(vp1[:], var[:], 1.0)
                rv = spool.tile([WL, HL], f32, tag="rv")
                nc.vector.reciprocal(rv[:], vp1[:])
                nalpha = spool.tile([WL, 1], f32, tag="nalpha")
                nc.vector.memset(nalpha[:], -ALPHA)
                cert = spool.tile([WL, HL], f32, tag="cert")
                nc.scalar.activation(cert[:], rv[:], AF.Exp,
                                     scale=ALPHA, bias=nalpha[:])
                ts = spool.tile([WL, HL], f32, tag="ts")
                nc.sync.dma_start(ts[:], t_scores[s])
                o1 = spool.tile([WL, HL], f32, tag="o1")
                nc.vector.tensor_tensor(out=o1[:], in0=cert[:], in1=mean[:],
                                        op=ALU.mult)
                o2 = spool.tile([WL, HL], f32, tag="o2")
                nc.vector.tensor_tensor(out=o2[:], in0=o1[:], in1=ts[:],
                                        op=ALU.add)
                nc.sync.dma_start(t_out[s], o2[:])

    nc.compile()
    return nc


def _get_nc():
    if "nc" not in _CACHE:
        _CACHE["nc"] = _build()
    return _CACHE["nc"]


def _bf16(a):
    import ml_dtypes
    return np.ascontiguousarray(a).astype(ml_dtypes.bfloat16)


def _run(test_scores, train_labels, test_feat, train_feats, softmax_temp,
         trace=False):
    from concourse.bass_utils import run_bass_kernel_spmd

    test_scores = np.ascontiguousarray(test_scores, np.float32)
    train_labels = np.ascontiguousarray(train_labels, np.float32)
    test_feat = np.ascontiguousarray(test_feat, np.float32)
    train_feats = np.ascontiguousarray(train_feats, np.float32)
    temp = np.ascontiguousarray(softmax_temp, np.float32).reshape(1)

    in_maps = []
    for c in range(NCORES):
        sl = slice(SEQ_LOC * c, SEQ_LOC * (c + 1))
        in_maps.append({
            "t_scores": test_scores[0, sl],
            "t_labels": np.ascontiguousarray(train_labels[:, sl]),
            "t_tefeat": test_feat[0, sl],
            "t_trfeat": np.ascontiguousarray(train_feats[:, sl]),
            "t_temp": temp,
            "t_dnrt": _bf16(_DN.T),
            "t_upt32": np.ascontiguousarray(_UP.T),
            "t_upt16": _bf16(_UP.T),
            "t_ident": _bf16(np.eye(WL, dtype=np.float32)),
        })
    nc = _get_nc()
    res = run_bass_kernel_spmd(nc, in_maps, list(range(NCORES)), trace=trace)
    out = np.empty((1, NSEQ, WL, HL), np.float32)
    for c in range(NCORES):
        out[0, SEQ_LOC * c:SEQ_LOC * (c + 1)] = res.results[c]["t_out"]
    return out, res


def kernel(test_scores, train_labels, test_feat, train_feats, softmax_temp):
    out, _ = _run(test_scores, train_labels, test_feat, train_feats,
                  softmax_temp, trace=False)
    return out


# revision 17
# speedup vs baseline: 5733.4509x; 1.1554x over previous
"""nn_AttentionAverageStdScalingModule — Trainium2 Bass/Tile kernel.

Contract: kernel(**inputs) takes FULL unsharded inputs and returns the FULL
output (1, 16, 88, 88) f32.  The nseq axis (16) is sharded 2-per-core across
8 NeuronCores; each core runs an identical program on its 2 sequences.

Per sequence s:
  te_n = softmax_temp * test_feat[:,s]/||cols||             (256, 484)
  for each memory m: simT[j,i] = sum_c tr[c,j]*te_n[c,i]    (484j, 484i)
      ez = exp(simT * rsqrt(nsq_j))   <- per-partition scale on ScalarE
      [num;den][i] = [labels_down_m; ones]^T @ ez           (2, 484)
  pmt_down = num/den; pmt = UP @ pmt_down @ UP^T  (bilinear upsample)
  mean/unbiased-std over m, certainty = exp(A/(1+std^2)-A)
  out = certainty*mean + test_scores

Engine split: PE does sim/aggregation/norm-sums/resampling matmuls (bf16),
ScalarE does only Exp (no LUT switches), VectorE does squares/stats and a
bit-trick rsqrt (Newton x2), GpSimd does the fp32->bf16 casts.  Memory-
sums land 4-memories-per-PSUM-bank at 32-aligned partitions so copies and
DRAM relayout bounces are batched.
"""

import numpy as np

NMEM, NSEQ, C, WF, HF = 30, 16, 256, 22, 22
WL, HL = 88, 88
P2 = WF * HF            # 484
NCORES = 8
SEQ_LOC = NSEQ // NCORES  # 2
ALPHA = 20.0
JC = [128, 128, 128, 100]   # j-chunk sizes covering 484
G4 = [list(range(4 * g, min(4 * g + 4, NMEM))) for g in range(8)]
# rsqrt batches (in units of g4 groups): ramp up so exp can start early
BATCHES = [[0], [1, 2], [3, 4], [5, 6, 7]]


def _resize_matrix(n_in: int, n_out: int) -> np.ndarray:
    """Row-stochastic 1-D bilinear resize matrix (half-pixel centers,
    out-of-range taps dropped + renormalized) matching
    jax.image.resize(method='bilinear', antialias=False)."""
    M = np.zeros((n_out, n_in), np.float64)
    scale = n_in / n_out
    for i in range(n_out):
        x = (i + 0.5) * scale - 0.5
        x0 = int(np.floor(x))
        for tap, w in ((x0, 1.0 - (x - x0)), (x0 + 1, x - x0)):
            if 0 <= tap < n_in and w > 0.0:
                M[i, tap] += w
        s = M[i].sum()
        if s > 0:
            M[i] /= s
    return M.astype(np.float32)


_DN = _resize_matrix(WL, WF)   # (22, 88)  downsample
_UP = _resize_matrix(WF, WL)   # (88, 22)  upsample

_CACHE = {}


def _build():
    import concourse.bacc as bacc
    import concourse.mybir as mybir
    from concourse import tile

    f32 = mybir.dt.float32
    bf16 = mybir.dt.bfloat16
    i32 = mybir.dt.int32
    AF = mybir.ActivationFunctionType
    ALU = mybir.AluOpType
    AX = mybir.AxisListType

    nc = bacc.Bacc("TRN2", target_bir_lowering=False, debug=False,
                   num_devices=NCORES)

    t_scores = nc.dram_tensor("t_scores", [SEQ_LOC, WL, HL], f32,
                              kind="ExternalInput")
    t_labels = nc.dram_tensor("t_labels", [NMEM, SEQ_LOC, WL, HL], f32,
                              kind="ExternalInput")
    t_tefeat = nc.dram_tensor("t_tefeat", [SEQ_LOC, C, WF, HF], f32,
                              kind="ExternalInput")
    t_trfeat = nc.dram_tensor("t_trfeat", [NMEM, SEQ_LOC, C, WF, HF], f32,
                              kind="ExternalInput")
    t_temp = nc.dram_tensor("t_temp", [1], f32, kind="ExternalInput")
    t_dnrt = nc.dram_tensor("t_dnrt", [WL, WF], bf16, kind="ExternalInput")
    t_upt32 = nc.dram_tensor("t_upt32", [WF, WL], f32, kind="ExternalInput")
    t_upt16 = nc.dram_tensor("t_upt16", [WF, WL], bf16, kind="ExternalInput")
    t_ident = nc.dram_tensor("t_ident", [WL, WL], bf16, kind="ExternalInput")
    t_out = nc.dram_tensor("t_out", [SEQ_LOC, WL, HL], f32,
                           kind="ExternalOutput")

    with tile.TileContext(nc) as tc:
        with (
            tc.tile_pool(name="const", bufs=1) as cpool,
            tc.tile_pool(name="seq", bufs=1) as spool,
            tc.tile_pool(name="big", bufs=3) as bpool,
            tc.tile_pool(name="ez", bufs=4) as ezpool,
            tc.tile_pool(name="sm", bufs=2) as smpool,
            tc.tile_pool(name="dram", bufs=1, space="DRAM") as dpool,
            tc.tile_pool(name="psA", bufs=4, space="PSUM") as psA,
            tc.tile_pool(name="psBC", bufs=2, space="PSUM") as psBC,
            tc.tile_pool(name="psD", bufs=1, space="PSUM") as psD,
        ):
            # ---- constants ----
            ones_col32 = cpool.tile([128, 1], f32)
            nc.vector.memset(ones_col32[:], 1.0)
            ones_col16 = cpool.tile([128, 1], bf16)
            nc.vector.memset(ones_col16[:], 1.0)
            ones_row32 = cpool.tile([1, 128], f32)
            nc.vector.memset(ones_row32[:], 1.0)
            dnrt = cpool.tile([WL, WF], bf16)      # DN^T (88, 22)
            nc.sync.dma_start(dnrt[:], t_dnrt[:])
            upt32 = cpool.tile([WF, WL], f32)      # UP^T (22, 88)
            nc.sync.dma_start(upt32[:], t_upt32[:])
            upt16 = cpool.tile([WF, WL], bf16)
            nc.sync.dma_start(upt16[:], t_upt16[:])
            ident = cpool.tile([WL, WL], bf16)
            nc.sync.dma_start(ident[:], t_ident[:])
            temp_t = cpool.tile([1, 1], f32)
            nc.sync.dma_start(temp_t[:], t_temp[:])

            for s in range(SEQ_LOC):
                # =========== test-feature prep ===========
                te32 = spool.tile([128, 2, P2], f32, tag="te32")
                nc.sync.dma_start(
                    te32[:],
                    t_tefeat[s].rearrange("(h p) w hh -> p h (w hh)", p=128))
                sq_te = spool.tile([128, 2, P2], f32, tag="sq_te")
                nc.vector.tensor_tensor(
                    out=sq_te[:], in0=te32[:], in1=te32[:], op=ALU.mult)
                nsq_te = psA.tile([1, P2], f32, tag="psa")
                for h in range(2):
                    nc.tensor.matmul(nsq_te[:], ones_col32[:], sq_te[:, h, :],
                                     start=(h == 0), stop=(h == 1))
                nrm_te = spool.tile([1, P2], f32, tag="nrm_te")
                nc.scalar.sqrt(nrm_te[:], nsq_te[:])
                inv_te = spool.tile([1, P2], f32, tag="inv_te")
                nc.vector.reciprocal(inv_te[:], nrm_te[:])
                tsc = spool.tile([1, P2], f32, tag="tsc")
                nc.vector.tensor_scalar_mul(tsc[:], inv_te[:],
                                            temp_t[0:1, 0:1])
                tsc_b = psA.tile([128, P2], f32, tag="psa")
                nc.tensor.matmul(tsc_b[:], ones_row32[:], tsc[:])
                te_n = spool.tile([128, 2, P2], bf16, tag="te_n")
                for h in range(2):
                    nc.vector.tensor_tensor(
                        out=te_n[:, h, :], in0=te32[:, h, :], in1=tsc_b[:],
                        op=ALU.mult)

                # =========== labels: bilinear 88->22, flatten j-major ======
                labs = spool.tile([WL, NMEM, HL], bf16, tag="labs")
                nc.gpsimd.dma_start(
                    labs[:], t_labels[:, s].rearrange("m w hh -> w m hh"))
                ld_stage = dpool.tile([NMEM, P2], bf16, tag="ld_stage")
                ldsb = spool.tile([WF, NMEM, WF], bf16, tag="ldsb")
                for m in range(NMEM):
                    b1 = psA.tile([WL, WF], f32, tag="psa")
                    nc.tensor.matmul(b1[:], labs[:, m, :], dnrt[:])
                    b1s = smpool.tile([WL, WF], bf16, tag="b1s")
                    nc.vector.tensor_copy(b1s[:], b1[:])
                    ldp = psA.tile([WF, WF], f32, tag="psa")
                    nc.tensor.matmul(ldp[:], b1s[:], dnrt[:])
                    nc.vector.tensor_copy(ldsb[:, m, :], ldp[:])
                nc.sync.dma_start(
                    ld_stage[:].rearrange("m (i k) -> i m k", k=WF), ldsb[:])
                ldj = spool.tile([128, NMEM, 4, 2], bf16, tag="ldj")
                nc.vector.memset(ldj[:], 0.0)
                nc.vector.memset(ldj[:, :, 0:3, 1], 1.0)
                nc.vector.memset(ldj[0:100, :, 3, 1], 1.0)
                for q in range(4):
                    pq = JC[q]
                    nc.sync.dma_start(
                        ldj[0:pq, :, q, 0],
                        ld_stage[:, 128 * q:128 * q + pq]
                        .rearrange("m p -> p m"))

                nrm_stage = dpool.tile([NMEM, 512], f32, tag="nrm_stage")
                nd_stage = dpool.tile([2, NMEM, P2], f32, tag="nd_stage")
                invj = spool.tile([128, NMEM, 4], f32, tag="invj")

                trbf_t = {}
                agg_state = {"agp": {}, "pending": None}
                # =========== main per-memory pipeline, in rsqrt batches ====
                for batch in BATCHES:
                    # -- phase 1: load + cast + squares + norm-sums --
                    for g in batch:
                        nsqp = psBC.tile([128, P2], f32, tag="psbc")
                        gm = G4[g]
                        pairs = [gm[i:i + 2] for i in range(0, len(gm), 2)]
                        sq_t = {}
                        for pr in pairs:
                            trp2 = bpool.tile([128, 2, 2, P2], bf16,
                                              tag="trbf", bufs=10)
                            for k, m in enumerate(pr):
                                nc.gpsimd.dma_start(
                                    trp2[:, k, :, :],
                                    t_trfeat[m, s]
                                    .rearrange("(h p) w hh -> p h (w hh)",
                                               p=128))
                                trbf_t[m] = trp2[:, k, :, :]
                            sq2 = bpool.tile([128, 2, 2, P2], bf16,
                                             tag="sqbf")
                            nc.vector.tensor_tensor(
                                out=sq2[:], in0=trp2[:], in1=trp2[:],
                                op=ALU.mult)
                            for k, m in enumerate(pr):
                                sq_t[m] = sq2[:, k, :, :]
                        for h in range(2):
                            for m in gm:
                                r = 32 * (m % 4)
                                nc.tensor.matmul(
                                    nsqp[r:r + 1, :], ones_col16[:],
                                    sq_t[m][:, h, :],
                                    start=(h == 0), stop=(h == 1),
                                    tile_position=(0, r))
                        nsqsb = smpool.tile([128, P2], f32, tag="nsqsb")
                        nc.vector.tensor_copy(nsqsb[:], nsqp[:])
                        m0 = G4[g][0]
                        nmg = len(G4[g])
                        nc.sync.dma_start(
                            nrm_stage[m0:m0 + nmg, 0:P2],
                            nsqsb[0:(nmg - 1) * 32 + 1:32, :])
                    # -- rsqrt for the whole batch (bit trick + 2 Newton) --
                    mlo = G4[batch[0]][0]
                    mhi = G4[batch[-1]][-1] + 1
                    nc.sync.dma_start(
                        invj[:, mlo:mhi, :],
                        nrm_stage[mlo:mhi].rearrange("m (q p) -> p m q",
                                                     p=128))
                    xv = invj[:, mlo:mhi, :]
                    xh = smpool.tile([128, NMEM, 4], f32, tag="xh")
                    nc.vector.tensor_scalar_mul(xh[:, mlo:mhi, :], xv, 0.5)
                    yv = smpool.tile([128, NMEM, 4], f32, tag="yv")
                    nc.vector.tensor_scalar(
                        out=yv[:, mlo:mhi, :].bitcast(i32),
                        in0=xv.bitcast(i32),
                        scalar1=1, scalar2=None,
                        op0=ALU.logical_shift_right)
                    nc.vector.tensor_scalar(
                        out=yv[:, mlo:mhi, :].bitcast(i32),
                        in0=yv[:, mlo:mhi, :].bitcast(i32),
                        scalar1=-1, scalar2=0x5F3759DF,
                        op0=ALU.mult, op1=ALU.add)
                    tv = smpool.tile([128, NMEM, 4], f32, tag="tv")
                    for _ in range(2):
                        nc.vector.tensor_tensor(
                            out=tv[:, mlo:mhi, :], in0=yv[:, mlo:mhi, :],
                            in1=yv[:, mlo:mhi, :], op=ALU.mult)
                        nc.vector.tensor_tensor(
                            out=tv[:, mlo:mhi, :], in0=tv[:, mlo:mhi, :],
                            in1=xh[:, mlo:mhi, :], op=ALU.mult)
                        nc.vector.tensor_scalar(
                            out=tv[:, mlo:mhi, :], in0=tv[:, mlo:mhi, :],
                            scalar1=-1.0, scalar2=1.5,
                            op0=ALU.mult, op1=ALU.add)
                        nc.vector.tensor_tensor(
                            out=yv[:, mlo:mhi, :], in0=yv[:, mlo:mhi, :],
                            in1=tv[:, mlo:mhi, :], op=ALU.mult)
                    nc.vector.tensor_copy(xv, yv[:, mlo:mhi, :])

                    # -- phase 2: sim + exp + aggregate (aggs lag
                    # one chunk so PE never waits on ScalarE) --
                    agp_t = agg_state["agp"]

                    def emit_aggs(g, q, ez_t):
                        pq = JC[q]
                        for m in G4[g]:
                            r = 32 * (m % 4)
                            nc.tensor.matmul(
                                agp_t[g][r:r + 2, :], ldj[0:pq, m, q, :],
                                ez_t[m][0:pq, :],
                                start=(q == 0), stop=(q == 3),
                                tile_position=(0, r))
                        if q == 3:
                            ndsb = smpool.tile([128, P2], f32, tag="ndsb")
                            nc.vector.tensor_copy(ndsb[:], agp_t[g][:])
                            m0 = G4[g][0]
                            nmg = len(G4[g])
                            nc.sync.dma_start(
                                nd_stage[0, m0:m0 + nmg, :],
                                ndsb[0:(nmg - 1) * 32 + 1:32, :])
                            nc.sync.dma_start(
                                nd_stage[1, m0:m0 + nmg, :],
                                ndsb[1:(nmg - 1) * 32 + 2:32, :])

                    for g in batch:
                        agp_t[g] = psBC.tile([128, P2], f32, tag="psbc",
                                             name=f"agp_{s}_{g}")
                        for q in range(4):
                            pq = JC[q]
                            j0 = 128 * q
                            ez_t = {}
                            for m in G4[g]:
                                trbf = trbf_t[m]
                                st = psA.tile([128, P2], f32, tag="psa")
                                for h in range(2):
                                    nc.tensor.matmul(
                                        st[0:pq, :],
                                        trbf[:, h, j0:j0 + pq],
                                        te_n[:, h, :],
                                        start=(h == 0), stop=(h == 1))
                                ez = ezpool.tile([128, P2], bf16, tag="ez",
                                                 bufs=10)
                                nc.scalar.activation(
                                    ez[0:pq, :], st[0:pq, :], AF.Exp,
                                    scale=invj[0:pq, m, q:q + 1])
                                ez_t[m] = ez
                            if agg_state["pending"] is not None:
                                emit_aggs(*agg_state["pending"])
                            agg_state["pending"] = (g, q, ez_t)
                        for m in G4[g]:
                            trbf_t.pop(m, None)
                if agg_state["pending"] is not None:
                    emit_aggs(*agg_state["pending"])
                    agg_state["pending"] = None

                # =========== batched num/den division ===========
                numt = spool.tile([121, 120], f32, tag="numt")
                nc.sync.dma_start(
                    numt[:], nd_stage[0].rearrange("m j -> (m j)")
                    .rearrange("(p x) -> p x", p=121))
                dent = spool.tile([121, 120], f32, tag="dent")
                nc.sync.dma_start(
                    dent[:], nd_stage[1].rearrange("m j -> (m j)")
                    .rearrange("(p x) -> p x", p=121))
                rden = spool.tile([121, 120], f32, tag="rden")
                nc.vector.reciprocal(rden[:], dent[:])
                pdq = spool.tile([121, 120], f32, tag="pdq")
                nc.vector.tensor_tensor(out=pdq[:], in0=numt[:], in1=rden[:],
                                        op=ALU.mult)
                pd_stage = dpool.tile([NMEM, P2], f32, tag="pd_stage")
                nc.sync.dma_start(
                    pd_stage[:].rearrange("m j -> (m j)")
                    .rearrange("(p x) -> p x", p=121), pdq[:])

                # =========== upsample + stats + output ===========
                xt = spool.tile([WF, NMEM * WF], f32, tag="xt")
                nc.sync.dma_start(
                    xt[:],
                    pd_stage[:].rearrange("m (i k) -> k (m i)", k=WF))
                d1a = psD.tile([WL, 512], f32, tag="d1a")
                nc.tensor.matmul(d1a[:], upt32[:], xt[:, 0:512])
                d1b = psD.tile([WL, NMEM * WF - 512], f32, tag="d1b")
                nc.tensor.matmul(d1b[:], upt32[:], xt[:, 512:])
                d1s = spool.tile([WL, NMEM, WF], bf16, tag="d1s")
                d1f = d1s[:].rearrange("l m j -> l (m j)")
                nc.vector.tensor_copy(d1f[:, 0:512], d1a[:])
                nc.vector.tensor_copy(d1f[:, 512:], d1b[:])
                d1t = spool.tile([WF, NMEM, WL], bf16, tag="d1t")
                for m in range(NMEM):
                    trp = psD.tile([WF, WL], bf16,
                                   tag=("d1a" if m % 2 else "d1b"))
                    nc.tensor.transpose(trp[:], d1s[:, m, :], ident[:])
                    nc.vector.tensor_copy(d1t[:, m, :], trp[:])

                s1 = spool.tile([WL, HL], f32, tag="s1")
                s2 = spool.tile([WL, HL], f32, tag="s2")
                for gg in range(6):
                    l0 = 16 * gg
                    nl = min(16, WL - l0)
                    d2 = psA.tile([WL, 480], f32, tag="psa")
                    nc.tensor.matmul(
                        d2[:, 0:nl * NMEM], upt16[:],
                        d1t[:].rearrange("j m l -> j l m")[:, l0:l0 + nl, :])
                    d2v = d2[:, 0:nl * NMEM].rearrange(
                        "i (l m) -> i l m", m=NMEM)
                    nc.vector.tensor_reduce(
                        s1[:, l0:l0 + nl], d2v, axis=AX.X, op=ALU.add)
                    sqg = smpool.tile([WL, 480], f32, tag="sqg")
                    nc.scalar.square(sqg[:, 0:nl * NMEM], d2[:, 0:nl * NMEM])
                    nc.vector.tensor_reduce(
                        s2[:, l0:l0 + nl],
                        sqg[:, 0:nl * NMEM].rearrange(
                            "i (l m) -> i l m", m=NMEM),
                        axis=AX.X, op=ALU.add)

                mean = spool.tile([WL, HL], f32, tag="mean")
                nc.vector.tensor_scalar_mul(mean[:], s1[:], 1.0 / NMEM)
                ms = spool.tile([WL, HL], f32, tag="ms")
                nc.vector.tensor_tensor(out=ms[:], in0=mean[:], in1=mean[:],
                                        op=ALU.mult)
                v1 = spool.tile([WL, HL], f32, tag="v1")
                nc.vector.tensor_scalar_mul(v1[:], s2[:], 1.0 / (NMEM - 1))
                v2 = spool.tile([WL, HL], f32, tag="v2")
                nc.vector.tensor_scalar_mul(v2[:], ms[:],
                                            NMEM / (NMEM - 1.0))
                var = spool.tile([WL, HL], f32, tag="var")
                nc.vector.tensor_tensor(out=var[:], in0=v1[:], in1=v2[:],
                                        op=ALU.subtract)
                vp1 = spool.tile([WL, HL], f32, tag="vp1")
                nc.vector.tensor_scalar_add(vp1[:], var[:], 1.0)
                rv = spool.tile([WL, HL], f32, tag="rv")
                nc.vector.reciprocal(rv[:], vp1[:])
                nalpha = spool.tile([WL, 1], f32, tag="nalpha")
                nc.vector.memset(nalpha[:], -ALPHA)
                cert = spool.tile([WL, HL], f32, tag="cert")
                nc.scalar.activation(cert[:], rv[:], AF.Exp,
                                     scale=ALPHA, bias=nalpha[:])
                ts = spool.tile([WL, HL], f32, tag="ts")
                nc.sync.dma_start(ts[:], t_scores[s])
                o1 = spool.tile([WL, HL], f32, tag="o1")
                nc.vector.tensor_tensor(out=o1[:], in0=cert[:], in1=mean[:],
                                        op=ALU.mult)
                o2 = spool.tile([WL, HL], f32, tag="o2")
                nc.vector.tensor_tensor(out=o2[:], in0=o1[:], in1=ts[:],
                                        op=ALU.add)
                nc.sync.dma_start(t_out[s], o2[:])

    nc.compile()
    return nc


def _get_nc():
    if "nc" not in _CACHE:
        _CACHE["nc"] = _build()
    return _CACHE["nc"]


def _bf16(a):
    import ml_dtypes
    return np.ascontiguousarray(a).astype(ml_dtypes.bfloat16)


def _run(test_scores, train_labels, test_feat, train_feats, softmax_temp,
         trace=False):
    from concourse.bass_utils import run_bass_kernel_spmd

    test_scores = np.ascontiguousarray(test_scores, np.float32)
    train_labels = np.ascontiguousarray(train_labels, np.float32)
    test_feat = np.ascontiguousarray(test_feat, np.float32)
    train_feats = np.ascontiguousarray(train_feats, np.float32)
    temp = np.ascontiguousarray(softmax_temp, np.float32).reshape(1)

    in_maps = []
    for c in range(NCORES):
        sl = slice(SEQ_LOC * c, SEQ_LOC * (c + 1))
        in_maps.append({
            "t_scores": test_scores[0, sl],
            "t_labels": np.ascontiguousarray(train_labels[:, sl]),
            "t_tefeat": test_feat[0, sl],
            "t_trfeat": np.ascontiguousarray(train_feats[:, sl]),
            "t_temp": temp,
            "t_dnrt": _bf16(_DN.T),
            "t_upt32": np.ascontiguousarray(_UP.T),
            "t_upt16": _bf16(_UP.T),
            "t_ident": _bf16(np.eye(WL, dtype=np.float32)),
        })
    nc = _get_nc()
    res = run_bass_kernel_spmd(nc, in_maps, list(range(NCORES)), trace=trace)
    out = np.empty((1, NSEQ, WL, HL), np.float32)
    for c in range(NCORES):
        out[0, SEQ_LOC * c:SEQ_LOC * (c + 1)] = res.results[c]["t_out"]
    return out, res


def kernel(test_scores, train_labels, test_feat, train_feats, softmax_temp):
    out, _ = _run(test_scores, train_labels, test_feat, train_feats,
                  softmax_temp, trace=False)
    return out


# revision 19
# speedup vs baseline: 6593.5930x; 1.1500x over previous
"""nn_AttentionAverageStdScalingModule — Trainium2 Bass/Tile kernel.

Contract: kernel(**inputs) takes FULL unsharded inputs and returns the FULL
output (1, 16, 88, 88) f32.  The nseq axis (16) is sharded 2-per-core across
8 NeuronCores; each core runs an identical program on its 2 sequences.

Per sequence s:
  te_n = softmax_temp * test_feat[:,s]/||cols||             (256, 484)
  for each memory m: simT[j,i] = sum_c tr[c,j]*te_n[c,i]    (484j, 484i)
      ez = exp(simT * rsqrt(nsq_j))   <- per-partition scale on ScalarE
      [num;den][i] = [labels_down_m; ones]^T @ ez           (2, 484)
  pmt_down = num/den; pmt = UP @ pmt_down @ UP^T  (bilinear upsample)
  mean/unbiased-std over m, certainty = exp(A/(1+std^2)-A)
  out = certainty*mean + test_scores

Engine split: PE does sim/aggregation/norm-sums/resampling matmuls (bf16),
ScalarE does only Exp (no LUT switches), VectorE does squares/stats and a
bit-trick rsqrt (Newton x2), GpSimd does the fp32->bf16 casts.  Memory-
sums land 4-memories-per-PSUM-bank at 32-aligned partitions so copies and
DRAM relayout bounces are batched.
"""

import numpy as np

NMEM, NSEQ, C, WF, HF = 30, 16, 256, 22, 22
WL, HL = 88, 88
P2 = WF * HF            # 484
NCORES = 8
SEQ_LOC = NSEQ // NCORES  # 2
ALPHA = 20.0
JC = [128, 128, 128, 100]   # j-chunk sizes covering 484
G4 = [list(range(4 * g, min(4 * g + 4, NMEM))) for g in range(8)]
# rsqrt batches (in units of g4 groups): ramp up so exp can start early
BATCHES = [[0], [1, 2], [3, 4], [5, 6, 7]]


def _resize_matrix(n_in: int, n_out: int) -> np.ndarray:
    """Row-stochastic 1-D bilinear resize matrix (half-pixel centers,
    out-of-range taps dropped + renormalized) matching
    jax.image.resize(method='bilinear', antialias=False)."""
    M = np.zeros((n_out, n_in), np.float64)
    scale = n_in / n_out
    for i in range(n_out):
        x = (i + 0.5) * scale - 0.5
        x0 = int(np.floor(x))
        for tap, w in ((x0, 1.0 - (x - x0)), (x0 + 1, x - x0)):
            if 0 <= tap < n_in and w > 0.0:
                M[i, tap] += w
        s = M[i].sum()
        if s > 0:
            M[i] /= s
    return M.astype(np.float32)


_DN = _resize_matrix(WL, WF)   # (22, 88)  downsample
_UP = _resize_matrix(WF, WL)   # (88, 22)  upsample

_CACHE = {}


def _build():
    import concourse.bacc as bacc
    import concourse.mybir as mybir
    from concourse import tile

    f32 = mybir.dt.float32
    bf16 = mybir.dt.bfloat16
    i32 = mybir.dt.int32
    AF = mybir.ActivationFunctionType
    ALU = mybir.AluOpType
    AX = mybir.AxisListType

    nc = bacc.Bacc("TRN2", target_bir_lowering=False, debug=False,
                   num_devices=NCORES)

    t_scores = nc.dram_tensor("t_scores", [SEQ_LOC, WL, HL], f32,
                              kind="ExternalInput")
    t_labels = nc.dram_tensor("t_labels", [NMEM, SEQ_LOC, WL, HL], f32,
                              kind="ExternalInput")
    t_tefeat = nc.dram_tensor("t_tefeat", [SEQ_LOC, C, WF, HF], f32,
                              kind="ExternalInput")
    t_trfeat = nc.dram_tensor("t_trfeat", [NMEM, SEQ_LOC, C, WF, HF], f32,
                              kind="ExternalInput")
    t_temp = nc.dram_tensor("t_temp", [1], f32, kind="ExternalInput")
    t_dnrt = nc.dram_tensor("t_dnrt", [WL, WF], bf16, kind="ExternalInput")
    t_upt32 = nc.dram_tensor("t_upt32", [WF, WL], f32, kind="ExternalInput")
    t_upt16 = nc.dram_tensor("t_upt16", [WF, WL], bf16, kind="ExternalInput")
    t_ident = nc.dram_tensor("t_ident", [WL, WL], bf16, kind="ExternalInput")
    t_out = nc.dram_tensor("t_out", [SEQ_LOC, WL, HL], f32,
                           kind="ExternalOutput")

    with tile.TileContext(nc) as tc:
        with (
            tc.tile_pool(name="const", bufs=1) as cpool,
            tc.tile_pool(name="seq", bufs=1) as spool,
            tc.tile_pool(name="big", bufs=3) as bpool,
            tc.tile_pool(name="ez", bufs=4) as ezpool,
            tc.tile_pool(name="sm", bufs=2) as smpool,
            tc.tile_pool(name="dram", bufs=1, space="DRAM") as dpool,
            tc.tile_pool(name="psA", bufs=4, space="PSUM") as psA,
            tc.tile_pool(name="psBC", bufs=2, space="PSUM") as psBC,
            tc.tile_pool(name="psD", bufs=1, space="PSUM") as psD,
        ):
            # ---- constants ----
            ones_col32 = cpool.tile([128, 1], f32)
            nc.vector.memset(ones_col32[:], 1.0)
            ones_col16 = cpool.tile([128, 1], bf16)
            nc.vector.memset(ones_col16[:], 1.0)
            ones_row32 = cpool.tile([1, 128], f32)
            nc.vector.memset(ones_row32[:], 1.0)
            dnrt = cpool.tile([WL, WF], bf16)      # DN^T (88, 22)
            nc.sync.dma_start(dnrt[:], t_dnrt[:])
            upt32 = cpool.tile([WF, WL], f32)      # UP^T (22, 88)
            nc.sync.dma_start(upt32[:], t_upt32[:])
            upt16 = cpool.tile([WF, WL], bf16)
            nc.sync.dma_start(upt16[:], t_upt16[:])
            ident = cpool.tile([WL, WL], bf16)
            nc.sync.dma_start(ident[:], t_ident[:])
            temp_t = cpool.tile([1, 1], f32)
            nc.sync.dma_start(temp_t[:], t_temp[:])

            te_n_s, ldj_s, invj_s = {}, {}, {}
            nrm_stage_s, nd_stage_s, pd_stage_s = {}, {}, {}
            mean_s, rv_s, ts_s = {}, {}, {}

            # =========== prep for BOTH sequences up front ===========
            for s in range(SEQ_LOC):
                te32 = spool.tile([128, 2, P2], f32, tag="te32")
                nc.sync.dma_start(
                    te32[:],
                    t_tefeat[s].rearrange("(h p) w hh -> p h (w hh)", p=128))
                sq_te = spool.tile([128, 2, P2], f32, tag="sq_te")
                nc.vector.tensor_tensor(
                    out=sq_te[:], in0=te32[:], in1=te32[:], op=ALU.mult)
                nsq_te = psA.tile([1, P2], f32, tag="psa")
                for h in range(2):
                    nc.tensor.matmul(nsq_te[:], ones_col32[:], sq_te[:, h, :],
                                     start=(h == 0), stop=(h == 1))
                nrm_te = spool.tile([1, P2], f32, tag="nrm_te")
                nc.scalar.sqrt(nrm_te[:], nsq_te[:])
                inv_te = spool.tile([1, P2], f32, tag="inv_te")
                nc.vector.reciprocal(inv_te[:], nrm_te[:])
                tsc = spool.tile([1, P2], f32, tag="tsc")
                nc.vector.tensor_scalar_mul(tsc[:], inv_te[:],
                                            temp_t[0:1, 0:1])
                tsc_b = psA.tile([128, P2], f32, tag="psa")
                nc.tensor.matmul(tsc_b[:], ones_row32[:], tsc[:])
                te_n = spool.tile([128, 2, P2], bf16, tag=f"te_n{s}")
                for h in range(2):
                    nc.vector.tensor_tensor(
                        out=te_n[:, h, :], in0=te32[:, h, :], in1=tsc_b[:],
                        op=ALU.mult)
                te_n_s[s] = te_n

                labs = spool.tile([WL, NMEM, HL], bf16, tag="labs")
                nc.gpsimd.dma_start(
                    labs[:], t_labels[:, s].rearrange("m w hh -> w m hh"))
                ld_stage = dpool.tile([NMEM, P2], bf16, tag=f"ld_stage{s}")
                ldsb = spool.tile([WF, NMEM, WF], bf16, tag="ldsb")
                for m in range(NMEM):
                    b1 = psA.tile([WL, WF], f32, tag="psa")
                    nc.tensor.matmul(b1[:], labs[:, m, :], dnrt[:])
                    b1s = smpool.tile([WL, WF], bf16, tag="b1s")
                    nc.vector.tensor_copy(b1s[:], b1[:])
                    ldp = psA.tile([WF, WF], f32, tag="psa")
                    nc.tensor.matmul(ldp[:], b1s[:], dnrt[:])
                    nc.vector.tensor_copy(ldsb[:, m, :], ldp[:])
                nc.sync.dma_start(
                    ld_stage[:].rearrange("m (i k) -> i m k", k=WF), ldsb[:])
                ldj = spool.tile([128, NMEM, 4, 2], bf16, tag=f"ldj{s}")
                nc.vector.memset(ldj[:], 0.0)
                nc.vector.memset(ldj[:, :, 0:3, 1], 1.0)
                nc.vector.memset(ldj[0:100, :, 3, 1], 1.0)
                for q in range(4):
                    pq = JC[q]
                    nc.sync.dma_start(
                        ldj[0:pq, :, q, 0],
                        ld_stage[:, 128 * q:128 * q + pq]
                        .rearrange("m p -> p m"))
                ldj_s[s] = ldj

                nrm_stage_s[s] = dpool.tile([NMEM, 512], f32,
                                            tag=f"nrm_stage{s}",
                                            name=f"nrm_stage{s}")
                nd_stage_s[s] = dpool.tile([2, NMEM, P2], f32,
                                           tag=f"nd_stage{s}",
                                           name=f"nd_stage{s}")
                invj_s[s] = spool.tile([128, NMEM, 4], f32, tag=f"invj{s}",
                                       name=f"invj{s}")
                ts = spool.tile([WL, HL], f32, tag=f"ts{s}")
                nc.sync.dma_start(ts[:], t_scores[s])
                ts_s[s] = ts

            # =========== main loops ===========
            for s in range(SEQ_LOC):
                te_n, ldj, invj = te_n_s[s], ldj_s[s], invj_s[s]
                nrm_stage, nd_stage = nrm_stage_s[s], nd_stage_s[s]
                trbf_t, sq_t = {}, {}
                pending = [None]
                agp_t = {}
                LA = 2

                def phase1(g):
                    nsqp = psBC.tile([128, P2], f32, tag="psbc",
                                     name=f"nsqp_{s}_{g}")
                    gm = G4[g]
                    pairs = [gm[i:i + 2] for i in range(0, len(gm), 2)]
                    for pr in pairs:
                        trp2 = bpool.tile([128, 2, 2, P2], bf16,
                                          tag="trbf", bufs=10,
                                          name=f"trp2_{s}_{pr[0]}")
                        for k, m in enumerate(pr):
                            nc.gpsimd.dma_start(
                                trp2[:, k, :, :],
                                t_trfeat[m, s]
                                .rearrange("(h p) w hh -> p h (w hh)",
                                           p=128))
                            trbf_t[m] = trp2[:, k, :, :]
                        sq2 = bpool.tile([128, 2, 2, P2], bf16, tag="sqbf",
                                         name=f"sq2_{s}_{pr[0]}")
                        nc.vector.tensor_tensor(
                            out=sq2[:], in0=trp2[:], in1=trp2[:],
                            op=ALU.mult)
                        for k, m in enumerate(pr):
                            sq_t[m] = sq2[:, k, :, :]
                    for h in range(2):
                        for m in gm:
                            r = 32 * (m % 4)
                            nc.tensor.matmul(
                                nsqp[r:r + 1, :], ones_col16[:],
                                sq_t[m][:, h, :],
                                start=(h == 0), stop=(h == 1),
                                tile_position=(0, r))
                    for m in gm:
                        sq_t.pop(m, None)
                    nsqsb = smpool.tile([128, P2], f32, tag="nsqsb")
                    nc.vector.tensor_copy(nsqsb[:], nsqp[:])
                    m0, nmg = gm[0], len(gm)
                    nc.sync.dma_start(
                        nrm_stage[m0:m0 + nmg, 0:P2],
                        nsqsb[0:(nmg - 1) * 32 + 1:32, :])
                    # readback + fast rsqrt (bit trick + 2 Newton steps)
                    nc.sync.dma_start(
                        invj[:, m0:m0 + nmg, :],
                        nrm_stage[m0:m0 + nmg].rearrange(
                            "m (q p) -> p m q", p=128))
                    xv = invj[:, m0:m0 + nmg, :]
                    nw = smpool.tile([128, 3, 4, 4], f32, tag="nw",
                                     name=f"nw_{s}_{g}")
                    xh = nw[:, 0, 0:nmg, :]
                    yv = nw[:, 1, 0:nmg, :]
                    tv = nw[:, 2, 0:nmg, :]
                    nc.vector.tensor_scalar_mul(xh, xv, 0.5)
                    nc.vector.tensor_scalar(
                        out=yv.bitcast(i32), in0=xv.bitcast(i32),
                        scalar1=1, scalar2=None,
                        op0=ALU.logical_shift_right)
                    nc.vector.tensor_scalar(
                        out=yv.bitcast(i32), in0=yv.bitcast(i32),
                        scalar1=-1, scalar2=0x5F3759DF,
                        op0=ALU.mult, op1=ALU.add)
                    for _ in range(2):
                        nc.vector.tensor_tensor(out=tv, in0=yv, in1=yv,
                                                op=ALU.mult)
                        nc.vector.tensor_tensor(out=tv, in0=tv, in1=xh,
                                                op=ALU.mult)
                        nc.vector.tensor_scalar(
                            out=tv, in0=tv, scalar1=-1.0, scalar2=1.5,
                            op0=ALU.mult, op1=ALU.add)
                        nc.vector.tensor_tensor(out=yv, in0=yv, in1=tv,
                                                op=ALU.mult)
                    nc.vector.tensor_copy(xv, yv)

                def emit_aggs(g, q, ez_t):
                    pq = JC[q]
                    for m in G4[g]:
                        r = 32 * (m % 4)
                        nc.tensor.matmul(
                            agp_t[g][r:r + 2, :], ldj[0:pq, m, q, :],
                            ez_t[m][0:pq, :],
                            start=(q == 0), stop=(q == 3),
                            tile_position=(0, r))
                    if q == 3:
                        ndsb = smpool.tile([128, P2], f32, tag="ndsb")
                        nc.vector.tensor_copy(ndsb[:], agp_t[g][:])
                        m0, nmg = G4[g][0], len(G4[g])
                        nc.sync.dma_start(
                            nd_stage[0, m0:m0 + nmg, :],
                            ndsb[0:(nmg - 1) * 32 + 1:32, :])
                        nc.sync.dma_start(
                            nd_stage[1, m0:m0 + nmg, :],
                            ndsb[1:(nmg - 1) * 32 + 2:32, :])

                def phase2(g):
                    agp_t[g] = psBC.tile([128, P2], f32, tag="psbc",
                                         name=f"agp_{s}_{g}")
                    for q in range(4):
                        pq = JC[q]
                        j0 = 128 * q
                        ez_t = {}
                        for m in G4[g]:
                            trbf = trbf_t[m]
                            st = psA.tile([128, P2], f32, tag="psa",
                                          name=f"st_{s}_{g}_{q}_{m}")
                            for h in range(2):
                                nc.tensor.matmul(
                                    st[0:pq, :],
                                    trbf[:, h, j0:j0 + pq],
                                    te_n[:, h, :],
                                    start=(h == 0), stop=(h == 1))
                            ez = ezpool.tile([128, P2], bf16, tag="ez",
                                             bufs=10,
                                             name=f"ez_{s}_{g}_{q}_{m}")
                            nc.scalar.activation(
                                ez[0:pq, :], st[0:pq, :], AF.Exp,
                                scale=invj[0:pq, m, q:q + 1])
                            ez_t[m] = ez
                        if pending[0] is not None:
                            emit_aggs(*pending[0])
                        pending[0] = (g, q, ez_t)
                    for m in G4[g]:
                        trbf_t.pop(m, None)

                NG = len(G4)
                for g in range(NG + LA):
                    if g < NG:
                        phase1(g)
                    if g >= LA:
                        phase2(g - LA)
                if pending[0] is not None:
                    emit_aggs(*pending[0])
                    pending[0] = None

                # ---- batched num/den division ----
                numt = spool.tile([121, 120], f32, tag="numt")
                nc.sync.dma_start(
                    numt[:], nd_stage[0].rearrange("m j -> (m j)")
                    .rearrange("(p x) -> p x", p=121))
                dent = spool.tile([121, 120], f32, tag="dent")
                nc.sync.dma_start(
                    dent[:], nd_stage[1].rearrange("m j -> (m j)")
                    .rearrange("(p x) -> p x", p=121))
                rden = spool.tile([121, 120], f32, tag="rden")
                nc.vector.reciprocal(rden[:], dent[:])
                pdq = spool.tile([121, 120], f32, tag="pdq")
                nc.vector.tensor_tensor(out=pdq[:], in0=numt[:], in1=rden[:],
                                        op=ALU.mult)
                pd_stage = dpool.tile([NMEM, P2], f32, tag=f"pd_stage{s}")
                nc.sync.dma_start(
                    pd_stage[:].rearrange("m j -> (m j)")
                    .rearrange("(p x) -> p x", p=121), pdq[:])

                # ---- upsample + stats (no ScalarE ops here) ----
                xt = spool.tile([WF, NMEM * WF], f32, tag="xt")
                nc.sync.dma_start(
                    xt[:],
                    pd_stage[:].rearrange("m (i k) -> k (m i)", k=WF))
                d1a = psD.tile([WL, 512], f32, tag="d1a")
                nc.tensor.matmul(d1a[:], upt32[:], xt[:, 0:512])
                d1b = psD.tile([WL, NMEM * WF - 512], f32, tag="d1b")
                nc.tensor.matmul(d1b[:], upt32[:], xt[:, 512:])
                d1s = spool.tile([WL, NMEM, WF], bf16, tag="d1s")
                d1f = d1s[:].rearrange("l m j -> l (m j)")
                nc.vector.tensor_copy(d1f[:, 0:512], d1a[:])
                nc.vector.tensor_copy(d1f[:, 512:], d1b[:])
                d1t = spool.tile([WF, NMEM, WL], bf16, tag="d1t")
                for m in range(NMEM):
                    trp = psD.tile([WF, WL], bf16,
                                   tag=("d1a" if m % 2 else "d1b"))
                    nc.tensor.transpose(trp[:], d1s[:, m, :], ident[:])
                    nc.vector.tensor_copy(d1t[:, m, :], trp[:])

                s1 = spool.tile([WL, HL], f32, tag="s1")
                s2 = spool.tile([WL, HL], f32, tag="s2")
                for gg in range(6):
                    l0 = 16 * gg
                    nl = min(16, WL - l0)
                    d2 = psA.tile([WL, 480], f32, tag="psa",
                                  name=f"d2_{s}_{gg}")
                    nc.tensor.matmul(
                        d2[:, 0:nl * NMEM], upt16[:],
                        d1t[:].rearrange("j m l -> j l m")[:, l0:l0 + nl, :])
                    d2c = smpool.tile([WL, 480], f32, tag="d2c")
                    nc.vector.tensor_copy(d2c[:, 0:nl * NMEM],
                                          d2[:, 0:nl * NMEM])
                    nc.vector.tensor_reduce(
                        s1[:, l0:l0 + nl],
                        d2c[:, 0:nl * NMEM].rearrange(
                            "i (l m) -> i l m", m=NMEM),
                        axis=AX.X, op=ALU.add)
                    sqg = smpool.tile([WL, 480], f32, tag="sqg")
                    nc.vector.tensor_tensor(
                        out=sqg[:, 0:nl * NMEM], in0=d2c[:, 0:nl * NMEM],
                        in1=d2c[:, 0:nl * NMEM], op=ALU.mult)
                    nc.vector.tensor_reduce(
                        s2[:, l0:l0 + nl],
                        sqg[:, 0:nl * NMEM].rearrange(
                            "i (l m) -> i l m", m=NMEM),
                        axis=AX.X, op=ALU.add)

                mean = spool.tile([WL, HL], f32, tag=f"mean{s}")
                nc.vector.tensor_scalar_mul(mean[:], s1[:], 1.0 / NMEM)
                ms = spool.tile([WL, HL], f32, tag="ms")
                nc.vector.tensor_tensor(out=ms[:], in0=mean[:], in1=mean[:],
                                        op=ALU.mult)
                v1 = spool.tile([WL, HL], f32, tag="v1")
                nc.vector.tensor_scalar_mul(v1[:], s2[:], 1.0 / (NMEM - 1))
                v2 = spool.tile([WL, HL], f32, tag="v2")
                nc.vector.tensor_scalar_mul(v2[:], ms[:],
                                            NMEM / (NMEM - 1.0))
                var = spool.tile([WL, HL], f32, tag="var")
                nc.vector.tensor_tensor(out=var[:], in0=v1[:], in1=v2[:],
                                        op=ALU.subtract)
                vp1 = spool.tile([WL, HL], f32, tag="vp1")
                nc.vector.tensor_scalar_add(vp1[:], var[:], 1.0)
                rv = spool.tile([WL, HL], f32, tag=f"rv{s}")
                nc.vector.reciprocal(rv[:], vp1[:])
                mean_s[s], rv_s[s] = mean, rv

            # =========== deferred certainty + output (after all exps) ======
            nalpha = cpool.tile([WL, 1], f32)
            nc.vector.memset(nalpha[:], -ALPHA)
            for s in range(SEQ_LOC):
                cert = spool.tile([WL, HL], f32, tag=f"cert{s}")
                nc.scalar.activation(cert[:], rv_s[s][:], AF.Exp,
                                     scale=ALPHA, bias=nalpha[:])
                o1 = spool.tile([WL, HL], f32, tag=f"o1{s}")
                nc.vector.tensor_tensor(out=o1[:], in0=cert[:],
                                        in1=mean_s[s][:], op=ALU.mult)
                o2 = spool.tile([WL, HL], f32, tag=f"o2{s}")
                nc.vector.tensor_tensor(out=o2[:], in0=o1[:], in1=ts_s[s][:],
                                        op=ALU.add)
                nc.sync.dma_start(t_out[s], o2[:])

    nc.compile()
    return nc


def _get_nc():
    if "nc" not in _CACHE:
        _CACHE["nc"] = _build()
    return _CACHE["nc"]


def _bf16(a):
    import ml_dtypes
    return np.ascontiguousarray(a).astype(ml_dtypes.bfloat16)


def _run(test_scores, train_labels, test_feat, train_feats, softmax_temp,
         trace=False):
    from concourse.bass_utils import run_bass_kernel_spmd

    test_scores = np.ascontiguousarray(test_scores, np.float32)
    train_labels = np.ascontiguousarray(train_labels, np.float32)
    test_feat = np.ascontiguousarray(test_feat, np.float32)
    train_feats = np.ascontiguousarray(train_feats, np.float32)
    temp = np.ascontiguousarray(softmax_temp, np.float32).reshape(1)

    in_maps = []
    for c in range(NCORES):
        sl = slice(SEQ_LOC * c, SEQ_LOC * (c + 1))
        in_maps.append({
            "t_scores": test_scores[0, sl],
            "t_labels": np.ascontiguousarray(train_labels[:, sl]),
            "t_tefeat": test_feat[0, sl],
            "t_trfeat": np.ascontiguousarray(train_feats[:, sl]),
            "t_temp": temp,
            "t_dnrt": _bf16(_DN.T),
            "t_upt32": np.ascontiguousarray(_UP.T),
            "t_upt16": _bf16(_UP.T),
            "t_ident": _bf16(np.eye(WL, dtype=np.float32)),
        })
    nc = _get_nc()
    res = run_bass_kernel_spmd(nc, in_maps, list(range(NCORES)), trace=trace)
    out = np.empty((1, NSEQ, WL, HL), np.float32)
    for c in range(NCORES):
        out[0, SEQ_LOC * c:SEQ_LOC * (c + 1)] = res.results[c]["t_out"]
    return out, res


def kernel(test_scores, train_labels, test_feat, train_feats, softmax_temp):
    out, _ = _run(test_scores, train_labels, test_feat, train_feats,
                  softmax_temp, trace=False)
    return out
